# revision 13
# baseline (speedup 1.0000x reference)
"""Trainium2 Bass kernel for nn_Encoder (S=4096, D=512, H=8, E=64).

Sharding: sequence-parallel over 8 cores. Each core computes the full K/V
(every query needs them) plus attention/MLP for its own 512 rows; the only
cross-core traffic is two 8-byte AllReduces for the global LayerNorm
statistics (the reference normalizes jointly over the whole [S, D] tensor).
The host concatenates the per-core row shards.

Per-core dataflow:
  - x^T tiles built with PE transposes; K^T [he, t] and V [t, he] follow as
    fp32r matmuls (two heads packed per 128-wide stationary), written to a
    DRAM scratch and streamed back during attention (SBUF can't hold both).
  - logits are computed transposed, L^T[t, q] = K^T-slice.T @ Q^T, so the
    Exp output is already the A@V moving operand; softmax denominators fall
    out of a ones-column appended to V (row 64 of the accumulator).
  - per-head tensors (Q^T, outH^T, own K^T/V^T) live at partitions 0..63
    with the head index on a free dim, so every matmul/DVE op sees matching
    base partitions.
  - the MLP uses h1^T = W1-slice.T @ out1^T so no intermediate needs an
    explicit transpose.
"""

import os

os.environ.setdefault("JAX_PLATFORMS", "axon")

import numpy as np

import concourse.bass as bass
import concourse.tile as tile
from concourse import mybir
from concourse.bass_utils import run_bass_kernel_spmd
from concourse.masks import make_identity

dt = mybir.dt
AF = mybir.ActivationFunctionType
ALU = mybir.AluOpType
AX = mybir.AxisListType

N_CORES = 8
S, D, H, E = 4096, 512, 8, 64
F = 4 * D          # 2048
R = S // N_CORES   # 512 rows per core
EPS = 1e-5
SCALE = 1.0 / float(np.sqrt(E))
INV_SD = 1.0 / float(S * D)


def split_waits(nc):
    """Walrus codegen allows only one sync-wait per HW instruction. Move
    extra waits onto single-wait NoOps inserted before, same engine queue."""
    import bass_rust

    n = 0
    for bb in nc.m.functions[0].blocks:
        new_list = []
        changed = False
        for ins in bb.instructions:
            si = ins.sync_info
            if si is not None and si.on_wait is not None and len(si.on_wait) > 1:
                waits = list(si.on_wait)
                for w in waits[:-1]:
                    nop = bass_rust.InstNoOp(name=f"I-xwait-{n}")
                    n += 1
                    nop.engine = ins.engine
                    nop.sync_info = bass_rust.SyncInfo(on_wait=[w], on_update=[])
                    nc.register_instruction(nop)
                    new_list.append(nop)
                si.on_wait = waits[-1:]
                ins.sync_info = si
                changed = True
            new_list.append(ins)
        if changed:
            bb.instructions = new_list
    return nc


def build_nc():
    import contextlib

    nc = bass.Bass("TRN2", debug=False, num_devices=N_CORES)
    f32, f32r = dt.float32, dt.float32r

    # ---- I/O ----------------------------------------------------------
    x_d = nc.dram_tensor("x", [S, D], f32, kind="ExternalInput").ap()
    Wq_d = nc.dram_tensor("Wq", [H, D, E], f32, kind="ExternalInput").ap()
    Wk_d = nc.dram_tensor("Wk", [H, D, E], f32, kind="ExternalInput").ap()
    Wv_d = nc.dram_tensor("Wv", [H, D, E], f32, kind="ExternalInput").ap()
    bq_d = nc.dram_tensor("bq", [H, E], f32, kind="ExternalInput").ap()
    bk_d = nc.dram_tensor("bk", [H, E], f32, kind="ExternalInput").ap()
    bv_d = nc.dram_tensor("bv", [H, E], f32, kind="ExternalInput").ap()
    Wo_d = nc.dram_tensor("Wo", [D, D], f32, kind="ExternalInput").ap()
    bo_d = nc.dram_tensor("bo", [D], f32, kind="ExternalInput").ap()
    W1_d = nc.dram_tensor("W1", [D, F], f32, kind="ExternalInput").ap()
    b1_d = nc.dram_tensor("b1", [F], f32, kind="ExternalInput").ap()
    W2_d = nc.dram_tensor("W2", [F, D], f32, kind="ExternalInput").ap()
    b2_d = nc.dram_tensor("b2", [D], f32, kind="ExternalInput").ap()
    xr_d = nc.dram_tensor("x_rows", [R, D], f32, kind="ExternalInput").ap()
    lng_d = nc.dram_tensor("ln_g_rows", [R, D], f32, kind="ExternalInput").ap()
    lnb_d = nc.dram_tensor("ln_b_rows", [R, D], f32, kind="ExternalInput").ap()

    fin_d = nc.dram_tensor("final_rows", [R, D], f32, kind="ExternalOutput").ap()
    kp_d = nc.dram_tensor("Kp_rows", [R, D], f32, kind="ExternalOutput").ap()
    vp_d = nc.dram_tensor("Vp_rows", [R, D], f32, kind="ExternalOutput").ap()

    # row index q = qc*128 + p everywhere
    xr_v = xr_d.rearrange("(c p) d -> p c d", p=128)
    lng_v = lng_d.rearrange("(c p) d -> p c d", p=128)
    lnb_v = lnb_d.rearrange("(c p) d -> p c d", p=128)
    fin_v = fin_d.rearrange("(c p) d -> p c d", p=128)
    kp_v = kp_d.rearrange("(c p) d -> p c d", p=128)
    vp_v = vp_d.rearrange("(c p) d -> p c d", p=128)

    with tile.TileContext(nc) as tc, contextlib.ExitStack() as ctx, \
            nc.allow_low_precision(reason="fp32r matmul operands (~tf32)"):
        ep = ctx.enter_context

        # ---- pools ----------------------------------------------------
        single = ep(tc.tile_pool(name="single", bufs=1))
        # a8: xa (phase-1 x tiles) -> xrT -> sq scratch      8KB x2
        a8 = ep(tc.tile_pool(name="a8", bufs=2))
        # big8: xt (x^T tiles) -> W1 quarters                8KB x4
        big8 = ep(tc.tile_pool(name="big8", bufs=4))
        # c8x: Wq/Wk/Wv -> W2 quarters                       8KB x4
        c8x = ep(tc.tile_pool(name="c8x", bufs=4))
        # d16: KTo/VTo [64,8,R] -> h1T halves [128,8,R]     16KB x2
        d16 = ep(tc.tile_pool(name="d16", bufs=2))
        # c8: xro(z), out1(w), out1T, fin                    8KB x2
        c8 = ep(tc.tile_pool(name="c8", bufs=2))
        qt_p = ep(tc.tile_pool(name="qt", bufs=1))   # Q^T [64, 8, R]
        ot_p = ep(tc.tile_pool(name="ot", bufs=1))   # outH^T [64, 8, R]
        evac = ep(tc.tile_pool(name="evac", bufs=2))
        pexp_p = ep(tc.tile_pool(name="pexp", bufs=2))
        kts_p = ep(tc.tile_pool(name="kts", bufs=2))
        vps_p = ep(tc.tile_pool(name="vps", bufs=2))
        otr_p = ep(tc.tile_pool(name="otr", bufs=1))
        ln_p = ep(tc.tile_pool(name="ln", bufs=1))
        wk = ep(tc.tile_pool(name="wk", bufs=1))
        # psum
        ps_tr = ep(tc.tile_pool(name="ps_tr", bufs=2, space="PSUM"))
        ps_mm = ep(tc.tile_pool(name="ps_mm", bufs=2, space="PSUM"))
        ps_l = ep(tc.tile_pool(name="ps_l", bufs=2, space="PSUM"))
        ps_oc = ep(tc.tile_pool(name="ps_oc", bufs=1, space="PSUM"))
        dram = ep(tc.tile_pool(name="dram", bufs=1, space="DRAM"))

        # DRAM scratch for K^T and V' (streamed back during attention)
        KT_dram = dram.tile([4, 128, S], f32r)            # [pair, he%128, t]
        VP_dram = dram.tile([32, 128, H, E + 1], f32r)    # [chunk, t%128, h, e']

        # ---- constants / small loads ---------------------------------
        ident = single.tile([128, 128], f32)
        make_identity(nc, ident[:])
        onesP = single.tile([128, 8], f32)
        nc.vector.memset(onesP[:], 1.0)
        ones1 = single.tile([1, 128], f32)
        nc.vector.memset(ones1[:], 1.0)
        ones_row = single.tile([1, 128], f32r)
        nc.vector.tensor_copy(ones_row[:], ones1[:])
        ones8 = single.tile([128, 8], f32r)
        nc.vector.tensor_copy(ones8[:], onesP[:])

        # per-head biases at partitions 0..63: [64 e, 8 h]
        bqs = single.tile([64, H], f32)
        bks = single.tile([64, H], f32)
        bvs = single.tile([64, H], f32)
        nc.sync.dma_start(bqs[:], bq_d.rearrange("h e -> e h"))
        nc.sync.dma_start(bks[:], bk_d.rearrange("h e -> e h"))
        nc.sync.dma_start(bvs[:], bv_d.rearrange("h e -> e h"))
        # packed-pair bias for the K^T evacuation: [(h%2)*64+e, h//2]
        bks2 = single.tile([128, 4], f32)
        nc.sync.dma_start(bks2[:], bk_d.rearrange("(c h2) e -> (h2 e) c", h2=2))
        b1s = single.tile([128, 16], f32)
        nc.sync.dma_start(b1s[:], b1_d.rearrange("(c p) -> p c", p=128))
        bo_r = single.tile([1, D], f32r)
        b2_r = single.tile([1, D], f32r)
        nc.gpsimd.dma_start(bo_r[:], bo_d.rearrange("(o d) -> o d", o=1))
        nc.gpsimd.dma_start(b2_r[:], b2_d.rearrange("(o d) -> o d", o=1))
        bv_bc = single.tile([128, D], f32)
        bv_flat = bv_d.rearrange("h e -> (h e)")
        nc.gpsimd.dma_start(
            bv_bc[:],
            bass.AP(tensor=bv_flat.tensor, offset=bv_flat.offset,
                    ap=[[0, 128]] + [list(a) for a in bv_flat.ap]),
        )
        eps_t = single.tile([1, 1], f32)
        nc.vector.memset(eps_t[:], EPS)

        # Wo in per-head-row layout: [64 e, 8 h, 512 dm]
        Wo_s = single.tile([64, H, D], f32r)
        nc.gpsimd.dma_start(Wo_s[:], Wo_d.rearrange("(h e) d -> e h d", e=E))

        # Wq/Wk/Wv as [p=d%128, dc, he] with he = (h//2)*128 + (h%2)*64 + e
        w_qkv = {}
        for name, wd in (("q", Wq_d), ("k", Wk_d), ("v", Wv_d)):
            t = c8x.tile([128, 4, D], f32r, tag="c8x")
            wv4 = wd.rearrange("h (dc p) e -> dc p h e", p=128)
            for dc in range(4):
                nc.gpsimd.dma_start(
                    t[:, dc, :].rearrange("p (h e) -> p h e", e=E), wv4[dc]
                )
            w_qkv[name] = t

        QT = qt_p.tile([64, H, R], f32r)            # Q^T + bq, per head
        xro = c8.tile([128, 4, D], f32, tag="c8")   # x own rows; becomes z
        nc.sync.dma_start(xro[:], xr_v)

        # ---- phase 1: x^T tiles -> K^T, V' (to DRAM scratch) ---------
        x_v = x_d.rearrange("(tt tb p) d -> tt p tb d", p=128, tb=4)
        for tt in range(8):
            xa = a8.tile([128, 4, D], f32, tag="a8")
            nc.sync.dma_start(xa[:], x_v[tt])
            xt = big8.tile([128, 4, 512], f32r, tag="big8")  # [d%128, dc, tl]
            for dc in range(4):
                for tb in range(4):
                    ptr = ps_tr.tile([128, 128], f32, tag="ptr")
                    nc.tensor.transpose(
                        ptr[:], xa[:, tb, dc * 128:(dc + 1) * 128], ident[:]
                    )
                    nc.vector.tensor_copy(
                        xt[:, dc, tb * 128:(tb + 1) * 128], ptr[:]
                    )
            for mc in range(4):
                pk = ps_mm.tile([128, 512], f32, tag="mm")
                for dc in range(4):
                    nc.tensor.matmul(
                        pk[:],
                        lhsT=w_qkv["k"][:, dc, mc * 128:(mc + 1) * 128],
                        rhs=xt[:, dc, :],
                        start=(dc == 0), stop=(dc == 3),
                    )
                ke = evac.tile([128, 512], f32r, tag="evac")
                nc.scalar.activation(
                    ke[:], pk[:], AF.Identity, bias=bks2[:, mc:mc + 1]
                )
                nc.sync.dma_start(KT_dram[mc, :, tt * 512:(tt + 1) * 512], ke[:])
            for vc in range(4):
                pv = ps_mm.tile([128, 512], f32, tag="mm")
                for dc in range(4):
                    nc.tensor.matmul(
                        pv[:],
                        lhsT=xt[:, dc, vc * 128:(vc + 1) * 128],
                        rhs=w_qkv["v"][:, dc, :],
                        start=(dc == 0), stop=(dc == 3),
                    )
                ve = evac.tile([128, H, E + 1], f32r, tag="evac")
                nc.vector.tensor_tensor(
                    ve[:, :, 0:E],
                    pv[:].rearrange("p (h e) -> p h e", e=E),
                    bv_bc[:].rearrange("p (h e) -> p h e", e=E),
                    ALU.add,
                )
                nc.vector.tensor_copy(ve[:, :, E], ones8[:])
                nc.sync.dma_start(VP_dram[tt * 4 + vc], ve[:])

        # ---- own-rows x^T, then per-head Q^T / K^T_own / V^T_own -----
        xrT = a8.tile([128, 4, R], f32r, tag="a8")
        for dc in range(4):
            for qc in range(4):
                ptr = ps_tr.tile([128, 128], f32, tag="ptr")
                nc.tensor.transpose(
                    ptr[:], xro[:, qc, dc * 128:(dc + 1) * 128], ident[:]
                )
                nc.vector.tensor_copy(xrT[:, dc, qc * 128:(qc + 1) * 128], ptr[:])

        def own_proj(dst, w_t, bias_t):
            """dst[64, h, R] = (x_rows @ W[h])^T + b[h], per head."""
            for h in range(H):
                he_local = (h // 2) * 128 + (h % 2) * 64
                pq = ps_mm.tile([64, 512], f32, tag="mm")
                for dc in range(4):
                    nc.tensor.matmul(
                        pq[:],
                        lhsT=w_t[:, dc, he_local:he_local + 64],
                        rhs=xrT[:, dc, :],
                        start=(dc == 0), stop=(dc == 3),
                    )
                nc.scalar.activation(
                    dst[:, h, :], pq[:], AF.Identity, bias=bias_t[:, h:h + 1]
                )

        def wo_project(src_T, out_view):
            """out_view rows = concat_h(src) @ Wo + bo (src per-head [64,8,R])."""
            for qc in range(4):
                po = ps_mm.tile([128, 512], f32, tag="mm")
                for h in range(H):
                    nc.tensor.matmul(
                        po[:],
                        lhsT=src_T[:, h, qc * 128:(qc + 1) * 128],
                        rhs=Wo_s[:, h, :],
                        start=(h == 0), stop=False,
                    )
                nc.tensor.matmul(
                    po[:], lhsT=ones_row[:], rhs=bo_r[:], start=False, stop=True
                )
                ot = evac.tile([128, 512], f32, tag="evac")
                nc.vector.tensor_copy(ot[:], po[:])
                nc.sync.dma_start(out_view[:, qc, :], ot[:])

        own_proj(QT, w_qkv["q"], bqs)
        KTo = d16.tile([64, H, R], f32r, tag="d16")
        own_proj(KTo, w_qkv["k"], bks)
        wo_project(KTo, kp_v)
        VTo = d16.tile([64, H, R], f32r, tag="d16")
        own_proj(VTo, w_qkv["v"], bvs)
        wo_project(VTo, vp_v)

        # ---- phase 2: attention --------------------------------------
        OT = ot_p.tile([64, H, R], f32r)   # normalized outH^T per head
        for h in range(H):
            hoff = (h % 2) * 64
            po = ps_oc.tile([E + 1, R], f32, tag="po_")
            for g in range(8):
                kt_s = kts_p.tile([64, 512], f32r, tag="kts")
                nc.sync.dma_start(
                    kt_s[:],
                    KT_dram[h // 2, hoff:hoff + 64, g * 512:(g + 1) * 512],
                )
                vp_s = vps_p.tile([128, 4, E + 1], f32r, tag="vps")
                nc.sync.dma_start(
                    vp_s[:],
                    VP_dram[g * 4:(g + 1) * 4, :, h, :].rearrange("c p e -> p c e"),
                )
                for cc in range(4):
                    pl = ps_l.tile([128, R], f32, tag="pl")
                    nc.tensor.matmul(
                        pl[:],
                        lhsT=kt_s[:, cc * 128:(cc + 1) * 128],
                        rhs=QT[:, h, :],
                        start=True, stop=True,
                    )
                    pexp = pexp_p.tile([128, R], f32r, tag="pexp")
                    nc.scalar.activation(pexp[:], pl[:], AF.Exp, scale=SCALE)
                    ch = g * 4 + cc
                    nc.tensor.matmul(
                        po[:],
                        lhsT=vp_s[:, cc, :],
                        rhs=pexp[:],
                        start=(ch == 0), stop=(ch == 31),
                    )
            # normalize rows 0..63 by the ones-column row 64
            otr = otr_p.tile([E + 1, R], f32, tag="otr")
            nc.scalar.copy(otr[:], po[:])
            rden = otr_p.tile([1, R], f32r, tag="rden")
            nc.vector.reciprocal(rden[:], otr[E:E + 1, :])
            pb = ps_oc.tile([E, R], f32, tag="sc")
            nc.tensor.matmul(
                pb[:], lhsT=ones_row[:, 0:E], rhs=rden[:], start=True, stop=True
            )
            nc.vector.tensor_tensor(OT[:, h, :], otr[0:E, :], pb[:], ALU.mult)

        # ---- phase 3: out proj + residual + global LN1 ---------------
        z = xro  # in place: z = x + out
        for qc in range(4):
            po = ps_mm.tile([128, 512], f32, tag="mm")
            for h in range(H):
                nc.tensor.matmul(
                    po[:],
                    lhsT=OT[:, h, qc * 128:(qc + 1) * 128],
                    rhs=Wo_s[:, h, :],
                    start=(h == 0), stop=False,
                )
            nc.tensor.matmul(
                po[:], lhsT=ones_row[:], rhs=bo_r[:], start=False, stop=True
            )
            nc.vector.tensor_tensor(z[:, qc, :], po[:], xro[:, qc, :], ALU.add)

        def global_ln_stats(src, tag):
            """-> [128, 2] sbuf tile: [:,0]=rstd, [:,1]=-mu*rstd (global)."""
            sums = wk.tile([128, 2], f32, tag=f"sums{tag}")
            nc.vector.tensor_reduce(
                out=sums[:, 0:1], in_=src[:], axis=AX.XY, op=ALU.add
            )
            sq = a8.tile([128, 4, D], f32, tag="a8")
            nc.scalar.activation(
                sq[:], src[:], AF.Square, accum_out=sums[:, 1:2]
            )
            pr = ps_oc.tile([1, 2], f32, tag="sc")
            nc.tensor.matmul(
                pr[:], lhsT=onesP[:, 0:1], rhs=sums[:], start=True, stop=True
            )
            part = wk.tile([1, 2], f32, tag=f"part{tag}")
            nc.vector.tensor_copy(part[:], pr[:])
            cin = dram.tile([1, 2], f32)
            cout = dram.tile([1, 2], f32)
            nc.sync.dma_start(cin[:], part[:])
            nc.gpsimd.collective_compute(
                "AllReduce", ALU.add,
                replica_groups=[list(range(N_CORES))],
                ins=[cin[:]], outs=[cout[:]],
            )
            tot = wk.tile([1, 2], f32, tag=f"tot{tag}")
            nc.sync.dma_start(tot[:], cout[:])
            sc = wk.tile([1, 6], f32, tag=f"sc{tag}")
            mu, m2 = sc[0:1, 0:1], sc[0:1, 1:2]
            nc.vector.tensor_scalar_mul(mu, tot[0:1, 0:1], INV_SD)
            nc.vector.tensor_scalar_mul(m2, tot[0:1, 1:2], INV_SD)
            nc.vector.tensor_tensor(sc[0:1, 2:3], mu, mu, ALU.mult)
            nc.vector.tensor_tensor(sc[0:1, 3:4], m2, sc[0:1, 2:3], ALU.subtract)
            nc.scalar.activation(sc[0:1, 4:5], sc[0:1, 3:4], AF.Sqrt, bias=eps_t[:])
            st2 = wk.tile([1, 2], f32r, tag=f"st2{tag}")
            nc.vector.reciprocal(st2[0:1, 0:1], sc[0:1, 4:5])        # rstd
            nc.vector.tensor_tensor(sc[0:1, 5:6], mu, st2[0:1, 0:1], ALU.mult)
            nc.vector.tensor_scalar_mul(st2[0:1, 1:2], sc[0:1, 5:6], -1.0)
            pbc = ps_oc.tile([128, 2], f32, tag="sc")
            nc.tensor.matmul(pbc[:], lhsT=ones_row[:], rhs=st2[:], start=True, stop=True)
            stb = wk.tile([128, 2], f32, tag=f"stb{tag}")
            nc.vector.tensor_copy(stb[:], pbc[:])
            return stb

        def ln_apply(dst_tile, src, stb, store_view=None):
            for qc in range(4):
                g_t = ln_p.tile([128, D], f32, tag="g")
                b_t = ln_p.tile([128, D], f32, tag="b")
                nc.sync.dma_start(g_t[:], lng_v[:, qc, :])
                nc.sync.dma_start(b_t[:], lnb_v[:, qc, :])
                n_t = evac.tile([128, D], f32, tag="evac")
                nc.scalar.activation(
                    n_t[:], src[:, qc, :], AF.Identity,
                    bias=stb[:, 1:2], scale=stb[:, 0:1],
                )
                nc.vector.tensor_tensor(n_t[:], n_t[:], g_t[:], ALU.mult)
                nc.vector.tensor_tensor(dst_tile[:, qc, :], n_t[:], b_t[:], ALU.add)
                if store_view is not None:
                    nc.sync.dma_start(store_view[:, qc, :], dst_tile[:, qc, :])

        stb1 = global_ln_stats(z, "a")
        out1 = c8.tile([128, 4, D], f32, tag="c8")
        ln_apply(out1, z, stb1)
        out1T = c8.tile([128, 4, R], f32r, tag="c8")
        for dc in range(4):
            for qc in range(4):
                ptr = ps_tr.tile([128, 128], f32, tag="ptr")
                nc.tensor.transpose(
                    ptr[:], out1[:, qc, dc * 128:(dc + 1) * 128], ident[:]
                )
                nc.vector.tensor_copy(out1T[:, dc, qc * 128:(qc + 1) * 128], ptr[:])

        # ---- phase 4: MLP + residual + global LN2 --------------------
        W1_v = W1_d.rearrange("(dc p) f -> dc p f", p=128)
        W1_s = []
        for j in range(4):
            t = big8.tile([128, F], f32r, tag="big8")
            nc.gpsimd.dma_start(t[:], W1_v[j])
            W1_s.append(t)
        W2_v = W2_d.rearrange("(g fc p) d -> g p fc d", p=128, fc=4)
        W2_s = []
        for j in range(4):
            t = c8x.tile([128, 4, D], f32r, tag="c8x")
            nc.gpsimd.dma_start(t[:], W2_v[j])
            W2_s.append(t)
        h1T = []
        for j in range(2):
            h1t_half = d16.tile([128, 8, R], f32r, tag="d16")
            h1T.append(h1t_half)
        for fm in range(16):
            ph = ps_mm.tile([128, R], f32, tag="mm")
            for dc in range(4):
                nc.tensor.matmul(
                    ph[:],
                    lhsT=W1_s[dc][:, fm * 128:(fm + 1) * 128],
                    rhs=out1T[:, dc, :],
                    start=(dc == 0), stop=(dc == 3),
                )
            nc.scalar.activation(
                h1T[fm // 8][:, fm % 8, :], ph[:], AF.Relu, bias=b1s[:, fm:fm + 1]
            )
        w = out1  # in place: w = out1 + out2
        for qc in range(4):
            po = ps_mm.tile([128, D], f32, tag="mm")
            for fm in range(16):
                nc.tensor.matmul(
                    po[:],
                    lhsT=h1T[fm // 8][:, fm % 8, qc * 128:(qc + 1) * 128],
                    rhs=W2_s[fm // 4][:, fm % 4, :],
                    start=(fm == 0), stop=False,
                )
            nc.tensor.matmul(
                po[:], lhsT=ones_row[:], rhs=b2_r[:], start=False, stop=True
            )
            nc.vector.tensor_tensor(w[:, qc, :], po[:], out1[:, qc, :], ALU.add)

        stb2 = global_ln_stats(w, "b")
        fin_s = c8.tile([128, 4, D], f32, tag="c8")
        ln_apply(fin_s, w, stb2, store_view=fin_v)

    split_waits(nc)
    return nc


_NC_CACHE = None


def _get_nc():
    global _NC_CACHE
    if _NC_CACHE is None:
        _NC_CACHE = build_nc()
    return _NC_CACHE


def kernel(**inputs):
    inp = {k: np.ascontiguousarray(np.asarray(v, dtype=np.float32))
           for k, v in inputs.items()}
    in_maps = []
    for c in range(N_CORES):
        rows = slice(c * R, (c + 1) * R)
        in_maps.append(dict(
            x=inp["x"], Wq=inp["Wq"], Wk=inp["Wk"], Wv=inp["Wv"],
            bq=inp["bq"], bk=inp["bk"], bv=inp["bv"],
            Wo=inp["Wo"], bo=inp["bo"], W1=inp["W1"], b1=inp["b1"],
            W2=inp["W2"], b2=inp["b2"],
            x_rows=inp["x"][rows],
            ln_g_rows=inp["ln_g"][rows], ln_b_rows=inp["ln_b"][rows],
        ))
    nc = _get_nc()
    res = run_bass_kernel_spmd(nc, in_maps, list(range(N_CORES)))
    final = np.concatenate([res.results[c]["final_rows"] for c in range(N_CORES)])
    Kp = np.concatenate([res.results[c]["Kp_rows"] for c in range(N_CORES)])
    Vp = np.concatenate([res.results[c]["Vp_rows"] for c in range(N_CORES)])
    return (final, Kp, Vp)


# revision 18
# speedup vs baseline: 1.1959x; 1.1959x over previous
"""Trainium2 Bass kernel for nn_Encoder (S=4096, D=512, H=8, E=64).

Sharding: sequence-parallel over 8 cores. Each core computes the full K/V
(every query needs them) plus attention/MLP for its own 512 rows; the only
cross-core traffic is two 8-byte AllReduces for the global LayerNorm
statistics (the reference normalizes jointly over the whole [S, D] tensor).
The host concatenates the per-core row shards.

Per-core dataflow:
  - x^T tiles built with PE transposes; K^T [he, t] and V [t, he] follow as
    fp32r matmuls (two heads packed per 128-wide stationary), written to a
    DRAM scratch and streamed back during attention (SBUF can't hold both).
  - logits are computed transposed, L^T[t, q] = K^T-slice.T @ Q^T, so the
    Exp output is already the A@V moving operand; softmax denominators fall
    out of a ones-column appended to V (row 64 of the accumulator).
  - per-head tensors (Q^T, outH^T, own K^T/V^T) live at partitions 0..63
    with the head index on a free dim, so every matmul/DVE op sees matching
    base partitions.
  - the MLP uses h1^T = W1-slice.T @ out1^T so no intermediate needs an
    explicit transpose.
"""

import os

os.environ.setdefault("JAX_PLATFORMS", "axon")

import numpy as np

import concourse.bass as bass
import concourse.tile as tile
from concourse import mybir
from concourse.bass_utils import run_bass_kernel_spmd
from concourse.masks import make_identity

dt = mybir.dt
AF = mybir.ActivationFunctionType
ALU = mybir.AluOpType
AX = mybir.AxisListType

N_CORES = 8
S, D, H, E = 4096, 512, 8, 64
F = 4 * D          # 2048
R = S // N_CORES   # 512 rows per core
EPS = 1e-5
SCALE = 1.0 / float(np.sqrt(E))
INV_SD = 1.0 / float(S * D)


def split_waits(nc):
    """Walrus codegen allows only one sync-wait per HW instruction. Move
    extra waits onto single-wait NoOps inserted before, same engine queue."""
    import bass_rust

    n = 0
    for bb in nc.m.functions[0].blocks:
        new_list = []
        changed = False
        for ins in bb.instructions:
            si = ins.sync_info
            if si is not None and si.on_wait is not None and len(si.on_wait) > 1:
                waits = list(si.on_wait)
                for w in waits[:-1]:
                    nop = bass_rust.InstNoOp(name=f"I-xwait-{n}")
                    n += 1
                    nop.engine = ins.engine
                    nop.sync_info = bass_rust.SyncInfo(on_wait=[w], on_update=[])
                    nc.register_instruction(nop)
                    new_list.append(nop)
                si.on_wait = waits[-1:]
                ins.sync_info = si
                changed = True
            new_list.append(ins)
        if changed:
            bb.instructions = new_list
    return nc


def build_nc():
    import contextlib

    nc = bass.Bass("TRN2", debug=False, num_devices=N_CORES)
    f32, f32r = dt.float32, dt.float32r

    # ---- I/O ----------------------------------------------------------
    x_d = nc.dram_tensor("x", [S, D], f32, kind="ExternalInput").ap()
    Wq_d = nc.dram_tensor("Wq", [H, D, E], f32, kind="ExternalInput").ap()
    Wk_d = nc.dram_tensor("Wk", [H, D, E], f32, kind="ExternalInput").ap()
    Wv_d = nc.dram_tensor("Wv", [H, D, E], f32, kind="ExternalInput").ap()
    bq_d = nc.dram_tensor("bq", [H, E], f32, kind="ExternalInput").ap()
    bk_d = nc.dram_tensor("bk", [H, E], f32, kind="ExternalInput").ap()
    bv_d = nc.dram_tensor("bv", [H, E], f32, kind="ExternalInput").ap()
    Wo_d = nc.dram_tensor("Wo", [D, D], f32, kind="ExternalInput").ap()
    bo_d = nc.dram_tensor("bo", [D], f32, kind="ExternalInput").ap()
    W1_d = nc.dram_tensor("W1", [D, F], f32, kind="ExternalInput").ap()
    b1_d = nc.dram_tensor("b1", [F], f32, kind="ExternalInput").ap()
    W2_d = nc.dram_tensor("W2", [F, D], f32, kind="ExternalInput").ap()
    b2_d = nc.dram_tensor("b2", [D], f32, kind="ExternalInput").ap()
    xr_d = nc.dram_tensor("x_rows", [R, D], f32, kind="ExternalInput").ap()
    lng_d = nc.dram_tensor("ln_g_rows", [R, D], f32, kind="ExternalInput").ap()
    lnb_d = nc.dram_tensor("ln_b_rows", [R, D], f32, kind="ExternalInput").ap()

    fin_d = nc.dram_tensor("final_rows", [R, D], f32, kind="ExternalOutput").ap()
    kp_d = nc.dram_tensor("Kp_rows", [R, D], f32, kind="ExternalOutput").ap()
    vp_d = nc.dram_tensor("Vp_rows", [R, D], f32, kind="ExternalOutput").ap()

    # row index q = qc*128 + p everywhere
    xr_v = xr_d.rearrange("(c p) d -> p c d", p=128)
    lng_v = lng_d.rearrange("(c p) d -> p c d", p=128)
    lnb_v = lnb_d.rearrange("(c p) d -> p c d", p=128)
    fin_v = fin_d.rearrange("(c p) d -> p c d", p=128)
    kp_v = kp_d.rearrange("(c p) d -> p c d", p=128)
    vp_v = vp_d.rearrange("(c p) d -> p c d", p=128)

    with tile.TileContext(nc) as tc, contextlib.ExitStack() as ctx, \
            nc.allow_low_precision(reason="bf16 matmul operands, fp32 accumulate"):
        ep = ctx.enter_context
        bf16 = dt.bfloat16

        # ---- pools ----------------------------------------------------
        single = ep(tc.tile_pool(name="single", bufs=1))
        a8 = ep(tc.tile_pool(name="a8", bufs=2))        # xa / xrT / sq
        big8 = ep(tc.tile_pool(name="big8", bufs=4))    # xt -> W1
        c8x = ep(tc.tile_pool(name="c8x", bufs=4))      # Wq/Wk/Wv -> W2
        d16 = ep(tc.tile_pool(name="d16", bufs=2))      # KTo/VTo -> h1T
        c8 = ep(tc.tile_pool(name="c8", bufs=2))        # xro(z), out1(w), out1T, fin
        qt_p = ep(tc.tile_pool(name="qt", bufs=1))      # Q^T [64, 8, R]
        ot_p = ep(tc.tile_pool(name="ot", bufs=1))      # outH^T [64, 8, R]
        evac = ep(tc.tile_pool(name="evac", bufs=4))
        pexp_p = ep(tc.tile_pool(name="pexp", bufs=3))
        kts_p = ep(tc.tile_pool(name="kts", bufs=6))
        vps_p = ep(tc.tile_pool(name="vps", bufs=3))
        otr_p = ep(tc.tile_pool(name="otr", bufs=2))
        ln_p = ep(tc.tile_pool(name="ln", bufs=2))
        wk = ep(tc.tile_pool(name="wk", bufs=2))
        # psum: tag "mm" 2x2banks + tag "po" 4x1bank = 8 banks
        ps_mm = ep(tc.tile_pool(name="ps_mm", bufs=2, space="PSUM"))
        ps_po = ep(tc.tile_pool(name="ps_po", bufs=4, space="PSUM"))
        dram = ep(tc.tile_pool(name="dram", bufs=1, space="DRAM"))

        # DRAM scratch for K^T and V' (streamed back during attention)
        KT_dram = dram.tile([H, 64, S], bf16)             # [h, e, t]
        VP_dram = dram.tile([32, 128, H, E + 1], bf16)    # [chunk, t%128, h, e']

        # ---- constants / small loads ---------------------------------
        ident = single.tile([128, 128], f32)
        make_identity(nc, ident[:])
        onesP = single.tile([128, 8], f32)
        nc.vector.memset(onesP[:], 1.0)
        ones1 = single.tile([1, 128], f32)
        nc.vector.memset(ones1[:], 1.0)
        ones_row = single.tile([1, 128], bf16)
        nc.vector.tensor_copy(ones_row[:], ones1[:])
        ones_row_r = single.tile([1, 128], f32r)
        nc.vector.tensor_copy(ones_row_r[:], ones1[:])
        ones8 = single.tile([128, 8], bf16)
        nc.vector.tensor_copy(ones8[:], onesP[:])

        # per-head biases at partitions 0..63: [64 e, 8 h]
        bqs = single.tile([64, H], f32)
        bks = single.tile([64, H], f32)
        bvs = single.tile([64, H], f32)
        nc.sync.dma_start(bqs[:], bq_d.rearrange("h e -> e h"))
        nc.sync.dma_start(bks[:], bk_d.rearrange("h e -> e h"))
        nc.sync.dma_start(bvs[:], bv_d.rearrange("h e -> e h"))
        # packed-pair bias for the K^T evacuation: [(h%2)*64+e, h//2]
        bks2 = single.tile([128, 4], f32)
        nc.sync.dma_start(bks2[:], bk_d.rearrange("(c h2) e -> (h2 e) c", h2=2))
        b1s = single.tile([128, 16], f32)
        nc.sync.dma_start(b1s[:], b1_d.rearrange("(c p) -> p c", p=128))
        bo_r = single.tile([1, D], bf16)
        b2_r = single.tile([1, D], bf16)
        nc.gpsimd.dma_start(bo_r[:], bo_d.rearrange("(o d) -> o d", o=1))
        nc.gpsimd.dma_start(b2_r[:], b2_d.rearrange("(o d) -> o d", o=1))
        bv_bc = single.tile([128, D], f32)
        bv_flat = bv_d.rearrange("h e -> (h e)")
        nc.gpsimd.dma_start(
            bv_bc[:],
            bass.AP(tensor=bv_flat.tensor, offset=bv_flat.offset,
                    ap=[[0, 128]] + [list(a) for a in bv_flat.ap]),
        )
        eps_t = single.tile([1, 1], f32)
        nc.vector.memset(eps_t[:], EPS)

        # Wo in per-head-row layout: [64 e, 8 h, 512 dm]
        Wo_s = single.tile([64, H, D], bf16)
        nc.gpsimd.dma_start(Wo_s[:], Wo_d.rearrange("(h e) d -> e h d", e=E))

        # Wq/Wk/Wv as [p=d%128, dc, he] with he = (h//2)*128 + (h%2)*64 + e
        w_qkv = {}
        for name, wd in (("q", Wq_d), ("k", Wk_d), ("v", Wv_d)):
            t = c8x.tile([128, 4, D], bf16, tag="c8x")
            wv4 = wd.rearrange("h (dc p) e -> dc p h e", p=128)
            for dc in range(4):
                nc.gpsimd.dma_start(
                    t[:, dc, :].rearrange("p (h e) -> p h e", e=E), wv4[dc]
                )
            w_qkv[name] = t

        QT = qt_p.tile([64, H, R], bf16)            # Q^T + bq, per head
        xro = c8.tile([128, 4, D], f32, tag="c8")   # x own rows; becomes z
        nc.sync.dma_start(xro[:], xr_v)

        # ---- phase 1: x^T tiles -> K^T, V' (to DRAM scratch) ---------
        x_v = x_d.rearrange("(tt tb p) d -> tt p tb d", p=128, tb=4)
        for tt in range(8):
            xa = a8.tile([128, 4, D], f32, tag="a8")
            nc.sync.dma_start(xa[:], x_v[tt])
            xt = big8.tile([128, 4, 512], bf16, tag="big8")  # [d%128, dc, tl]
            for dc in range(4):
                for tb in range(4):
                    ptr = ps_po.tile([128, 128], f32, tag="po")
                    nc.tensor.transpose(
                        ptr[:], xa[:, tb, dc * 128:(dc + 1) * 128], ident[:]
                    )
                    nc.vector.tensor_copy(
                        xt[:, dc, tb * 128:(tb + 1) * 128], ptr[:]
                    )
            for mc in range(4):
                pk = ps_mm.tile([128, 512], f32, tag="mm")
                for dc in range(4):
                    nc.tensor.matmul(
                        pk[:],
                        lhsT=w_qkv["k"][:, dc, mc * 128:(mc + 1) * 128],
                        rhs=xt[:, dc, :],
                        start=(dc == 0), stop=(dc == 3),
                    )
                ke = evac.tile([128, 512], bf16, tag="evac")
                nc.scalar.activation(
                    ke[:], pk[:], AF.Identity, bias=bks2[:, mc:mc + 1]
                )
                nc.sync.dma_start(
                    KT_dram[2 * mc, :, tt * 512:(tt + 1) * 512], ke[0:64, :]
                )
                nc.sync.dma_start(
                    KT_dram[2 * mc + 1, :, tt * 512:(tt + 1) * 512], ke[64:128, :]
                )
            for vc in range(4):
                pv = ps_mm.tile([128, 512], f32, tag="mm")
                for dc in range(4):
                    nc.tensor.matmul(
                        pv[:],
                        lhsT=xt[:, dc, vc * 128:(vc + 1) * 128],
                        rhs=w_qkv["v"][:, dc, :],
                        start=(dc == 0), stop=(dc == 3),
                    )
                ve = evac.tile([128, H, E + 1], bf16, tag="evac")
                nc.vector.tensor_tensor(
                    ve[:, :, 0:E],
                    pv[:].rearrange("p (h e) -> p h e", e=E),
                    bv_bc[:].rearrange("p (h e) -> p h e", e=E),
                    ALU.add,
                )
                nc.vector.tensor_copy(ve[:, :, E], ones8[:])
                nc.sync.dma_start(VP_dram[tt * 4 + vc], ve[:])

        # ---- own-rows x^T, then per-head Q^T / K^T_own / V^T_own -----
        xrT = a8.tile([128, 4, R], bf16, tag="a8")
        for dc in range(4):
            for qc in range(4):
                ptr = ps_po.tile([128, 128], f32, tag="po")
                nc.tensor.transpose(
                    ptr[:], xro[:, qc, dc * 128:(dc + 1) * 128], ident[:]
                )
                nc.vector.tensor_copy(xrT[:, dc, qc * 128:(qc + 1) * 128], ptr[:])

        def own_proj(dst, w_t, bias_t):
            """dst[64, h, R] = (x_rows @ W[h])^T + b[h], per head."""
            for h in range(H):
                he_local = (h // 2) * 128 + (h % 2) * 64
                pq = ps_mm.tile([64, 512], f32, tag="mm")
                for dc in range(4):
                    nc.tensor.matmul(
                        pq[:],
                        lhsT=w_t[:, dc, he_local:he_local + 64],
                        rhs=xrT[:, dc, :],
                        start=(dc == 0), stop=(dc == 3),
                    )
                nc.scalar.activation(
                    dst[:, h, :], pq[:], AF.Identity, bias=bias_t[:, h:h + 1]
                )

        def wo_project(src_T, out_view):
            """out_view rows = concat_h(src) @ Wo + bo (src per-head [64,8,R])."""
            for qc in range(4):
                po = ps_mm.tile([128, 512], f32, tag="mm")
                for h in range(H):
                    nc.tensor.matmul(
                        po[:],
                        lhsT=src_T[:, h, qc * 128:(qc + 1) * 128],
                        rhs=Wo_s[:, h, :],
                        start=(h == 0), stop=False,
                    )
                nc.tensor.matmul(
                    po[:], lhsT=ones_row[:], rhs=bo_r[:], start=False, stop=True
                )
                ot = evac.tile([128, 512], f32, tag="evac")
                nc.vector.tensor_copy(ot[:], po[:])
                nc.sync.dma_start(out_view[:, qc, :], ot[:])

        own_proj(QT, w_qkv["q"], bqs)
        KTo = d16.tile([64, H, R], bf16, tag="d16")
        own_proj(KTo, w_qkv["k"], bks)
        wo_project(KTo, kp_v)
        VTo = d16.tile([64, H, R], bf16, tag="d16")
        own_proj(VTo, w_qkv["v"], bvs)
        wo_project(VTo, vp_v)

        # ---- phase 2: attention (2 passes x 4 heads interleaved) -----
        OT = ot_p.tile([64, H, R], bf16)   # normalized outH^T per head
        for pass_ in range(2):
            heads = [4 * pass_ + i for i in range(4)]
            po4 = []
            for hh in range(4):
                po_t = ps_po.tile([E + 1, R], f32, tag="po")
                po4.append(po_t)
            for g in range(8):
                kt4 = []
                for hh, h in enumerate(heads):
                    kt_s = kts_p.tile([64, 512], bf16, tag="kts")
                    nc.sync.dma_start(
                        kt_s[:], KT_dram[h, :, g * 512:(g + 1) * 512]
                    )
                    kt4.append(kt_s)
                for cc in range(4):
                    ch = g * 4 + cc
                    vf = vps_p.tile([128, H, E + 1], bf16, tag="vps")
                    nc.sync.dma_start(vf[:], VP_dram[ch])
                    for sub in range(2):
                        h0, h1 = heads[2 * sub], heads[2 * sub + 1]
                        pl = ps_mm.tile([128, 2, 512], f32, tag="mm")
                        nc.tensor.matmul(
                            pl[:, 0, :],
                            lhsT=kt4[2 * sub][:, cc * 128:(cc + 1) * 128],
                            rhs=QT[:, h0, :], start=True, stop=True,
                        )
                        nc.tensor.matmul(
                            pl[:, 1, :],
                            lhsT=kt4[2 * sub + 1][:, cc * 128:(cc + 1) * 128],
                            rhs=QT[:, h1, :], start=True, stop=True,
                        )
                        pexp = pexp_p.tile([128, 2, 512], bf16, tag="pexp")
                        nc.scalar.activation(pexp[:], pl[:], AF.Exp, scale=SCALE)
                        nc.tensor.matmul(
                            po4[2 * sub][:],
                            lhsT=vf[:, h0, :], rhs=pexp[:, 0, :],
                            start=(ch == 0), stop=(ch == 31),
                        )
                        nc.tensor.matmul(
                            po4[2 * sub + 1][:],
                            lhsT=vf[:, h1, :], rhs=pexp[:, 1, :],
                            start=(ch == 0), stop=(ch == 31),
                        )
            # normalize rows 0..63 by the ones-column row 64
            for hh, h in enumerate(heads):
                otr = otr_p.tile([E + 1, R], f32, tag="otr")
                nc.scalar.copy(otr[:], po4[hh][:])
                rden = otr_p.tile([1, R], f32r, tag="rden")
                nc.vector.reciprocal(rden[:], otr[E:E + 1, :])
                pb = ps_mm.tile([E, R], f32, tag="mm")
                nc.tensor.matmul(
                    pb[:], lhsT=ones_row_r[:, 0:E], rhs=rden[:],
                    start=True, stop=True,
                )
                nc.vector.tensor_tensor(OT[:, h, :], otr[0:E, :], pb[:], ALU.mult)

        # ---- phase 3: out proj + residual + global LN1 ---------------
        z = xro  # in place: z = x + out
        for qc in range(4):
            po = ps_mm.tile([128, 512], f32, tag="mm")
            for h in range(H):
                nc.tensor.matmul(
                    po[:],
                    lhsT=OT[:, h, qc * 128:(qc + 1) * 128],
                    rhs=Wo_s[:, h, :],
                    start=(h == 0), stop=False,
                )
            nc.tensor.matmul(
                po[:], lhsT=ones_row[:], rhs=bo_r[:], start=False, stop=True
            )
            nc.vector.tensor_tensor(z[:, qc, :], po[:], xro[:, qc, :], ALU.add)

        def global_ln_stats(src, tag):
            """-> [128, 2] sbuf tile: [:,0]=rstd, [:,1]=-mu*rstd (global)."""
            sums = wk.tile([128, 2], f32, tag=f"sums{tag}")
            nc.vector.tensor_reduce(
                out=sums[:, 0:1], in_=src[:], axis=AX.XY, op=ALU.add
            )
            sq = a8.tile([128, 4, D], f32, tag="a8")
            nc.scalar.activation(
                sq[:], src[:], AF.Square, accum_out=sums[:, 1:2]
            )
            pr = ps_po.tile([1, 2], f32, tag="po")
            nc.tensor.matmul(
                pr[:], lhsT=onesP[:, 0:1], rhs=sums[:], start=True, stop=True
            )
            part = wk.tile([1, 2], f32, tag=f"part{tag}")
            nc.vector.tensor_copy(part[:], pr[:])
            cin = dram.tile([1, 2], f32)
            cout = dram.tile([1, 2], f32)
            nc.sync.dma_start(cin[:], part[:])
            nc.gpsimd.collective_compute(
                "AllReduce", ALU.add,
                replica_groups=[list(range(N_CORES))],
                ins=[cin[:]], outs=[cout[:]],
            )
            tot = wk.tile([1, 2], f32, tag=f"tot{tag}")
            nc.sync.dma_start(tot[:], cout[:])
            sc = wk.tile([1, 6], f32, tag=f"sc{tag}")
            mu, m2 = sc[0:1, 0:1], sc[0:1, 1:2]
            nc.vector.tensor_scalar_mul(mu, tot[0:1, 0:1], INV_SD)
            nc.vector.tensor_scalar_mul(m2, tot[0:1, 1:2], INV_SD)
            nc.vector.tensor_tensor(sc[0:1, 2:3], mu, mu, ALU.mult)
            nc.vector.tensor_tensor(sc[0:1, 3:4], m2, sc[0:1, 2:3], ALU.subtract)
            nc.scalar.activation(sc[0:1, 4:5], sc[0:1, 3:4], AF.Sqrt, bias=eps_t[:])
            st2 = wk.tile([1, 2], f32r, tag=f"st2{tag}")
            nc.vector.reciprocal(st2[0:1, 0:1], sc[0:1, 4:5])        # rstd
            nc.vector.tensor_tensor(sc[0:1, 5:6], mu, st2[0:1, 0:1], ALU.mult)
            nc.vector.tensor_scalar_mul(st2[0:1, 1:2], sc[0:1, 5:6], -1.0)
            pbc = ps_po.tile([128, 2], f32, tag="po")
            nc.tensor.matmul(pbc[:], lhsT=ones_row_r[:], rhs=st2[:], start=True, stop=True)
            stb = wk.tile([128, 2], f32, tag=f"stb{tag}")
            nc.vector.tensor_copy(stb[:], pbc[:])
            return stb

        def ln_apply(dst_tile, src, stb, store_view=None):
            for qc in range(4):
                g_t = ln_p.tile([128, D], f32, tag="g")
                b_t = ln_p.tile([128, D], f32, tag="b")
                nc.sync.dma_start(g_t[:], lng_v[:, qc, :])
                nc.sync.dma_start(b_t[:], lnb_v[:, qc, :])
                n_t = evac.tile([128, D], f32, tag="evac")
                nc.scalar.activation(
                    n_t[:], src[:, qc, :], AF.Identity,
                    bias=stb[:, 1:2], scale=stb[:, 0:1],
                )
                nc.vector.tensor_tensor(n_t[:], n_t[:], g_t[:], ALU.mult)
                nc.vector.tensor_tensor(dst_tile[:, qc, :], n_t[:], b_t[:], ALU.add)
                if store_view is not None:
                    nc.sync.dma_start(store_view[:, qc, :], dst_tile[:, qc, :])

        stb1 = global_ln_stats(z, "a")
        out1 = c8.tile([128, 4, D], f32, tag="c8")
        ln_apply(out1, z, stb1)
        out1T = c8.tile([128, 4, R], bf16, tag="c8")
        for dc in range(4):
            for qc in range(4):
                ptr = ps_po.tile([128, 128], f32, tag="po")
                nc.tensor.transpose(
                    ptr[:], out1[:, qc, dc * 128:(dc + 1) * 128], ident[:]
                )
                nc.vector.tensor_copy(out1T[:, dc, qc * 128:(qc + 1) * 128], ptr[:])

        # ---- phase 4: MLP + residual + global LN2 --------------------
        W1_v = W1_d.rearrange("(dc p) f -> dc p f", p=128)
        W1_s = []
        for j in range(4):
            t = big8.tile([128, F], bf16, tag="big8")
            nc.gpsimd.dma_start(t[:], W1_v[j])
            W1_s.append(t)
        W2_v = W2_d.rearrange("(g fc p) d -> g p fc d", p=128, fc=4)
        W2_s = []
        for j in range(4):
            t = c8x.tile([128, 4, D], bf16, tag="c8x")
            nc.gpsimd.dma_start(t[:], W2_v[j])
            W2_s.append(t)
        h1T = []
        for j in range(2):
            h1t_half = d16.tile([128, 8, R], bf16, tag="d16")
            h1T.append(h1t_half)
        for fm in range(16):
            ph = ps_mm.tile([128, R], f32, tag="mm")
            for dc in range(4):
                nc.tensor.matmul(
                    ph[:],
                    lhsT=W1_s[dc][:, fm * 128:(fm + 1) * 128],
                    rhs=out1T[:, dc, :],
                    start=(dc == 0), stop=(dc == 3),
                )
            nc.scalar.activation(
                h1T[fm // 8][:, fm % 8, :], ph[:], AF.Relu, bias=b1s[:, fm:fm + 1]
            )
        w = out1  # in place: w = out1 + out2
        for qc in range(4):
            po = ps_mm.tile([128, D], f32, tag="mm")
            for fm in range(16):
                nc.tensor.matmul(
                    po[:],
                    lhsT=h1T[fm // 8][:, fm % 8, qc * 128:(qc + 1) * 128],
                    rhs=W2_s[fm // 4][:, fm % 4, :],
                    start=(fm == 0), stop=False,
                )
            nc.tensor.matmul(
                po[:], lhsT=ones_row[:], rhs=b2_r[:], start=False, stop=True
            )
            nc.vector.tensor_tensor(w[:, qc, :], po[:], out1[:, qc, :], ALU.add)

        stb2 = global_ln_stats(w, "b")
        fin_s = c8.tile([128, 4, D], f32, tag="c8")
        ln_apply(fin_s, w, stb2, store_view=fin_v)

    split_waits(nc)
    return nc


_NC_CACHE = None


def _get_nc():
    global _NC_CACHE
    if _NC_CACHE is None:
        _NC_CACHE = build_nc()
    return _NC_CACHE


def kernel(**inputs):
    inp = {k: np.ascontiguousarray(np.asarray(v, dtype=np.float32))
           for k, v in inputs.items()}
    in_maps = []
    for c in range(N_CORES):
        rows = slice(c * R, (c + 1) * R)
        in_maps.append(dict(
            x=inp["x"], Wq=inp["Wq"], Wk=inp["Wk"], Wv=inp["Wv"],
            bq=inp["bq"], bk=inp["bk"], bv=inp["bv"],
            Wo=inp["Wo"], bo=inp["bo"], W1=inp["W1"], b1=inp["b1"],
            W2=inp["W2"], b2=inp["b2"],
            x_rows=inp["x"][rows],
            ln_g_rows=inp["ln_g"][rows], ln_b_rows=inp["ln_b"][rows],
        ))
    nc = _get_nc()
    res = run_bass_kernel_spmd(nc, in_maps, list(range(N_CORES)))
    final = np.concatenate([res.results[c]["final_rows"] for c in range(N_CORES)])
    Kp = np.concatenate([res.results[c]["Kp_rows"] for c in range(N_CORES)])
    Vp = np.concatenate([res.results[c]["Vp_rows"] for c in range(N_CORES)])
    return (final, Kp, Vp)


# revision 23
# speedup vs baseline: 1.2790x; 1.0694x over previous
"""Trainium2 Bass kernel for nn_Encoder (S=4096, D=512, H=8, E=64).

Sharding: sequence-parallel over 8 cores. Each core computes the full K/V
(every query needs them) plus attention/MLP for its own 512 rows; the only
cross-core traffic is two 8-byte AllReduces for the global LayerNorm
statistics (the reference normalizes jointly over the whole [S, D] tensor).
The host concatenates the per-core row shards.

Per-core dataflow:
  - x^T tiles built with PE transposes; K^T [he, t] and V [t, he] follow as
    fp32r matmuls (two heads packed per 128-wide stationary), written to a
    DRAM scratch and streamed back during attention (SBUF can't hold both).
  - logits are computed transposed, L^T[t, q] = K^T-slice.T @ Q^T, so the
    Exp output is already the A@V moving operand; softmax denominators fall
    out of a ones-column appended to V (row 64 of the accumulator).
  - per-head tensors (Q^T, outH^T, own K^T/V^T) live at partitions 0..63
    with the head index on a free dim, so every matmul/DVE op sees matching
    base partitions.
  - the MLP uses h1^T = W1-slice.T @ out1^T so no intermediate needs an
    explicit transpose.
"""

import os

os.environ.setdefault("JAX_PLATFORMS", "axon")

import numpy as np

import concourse.bass as bass
import concourse.tile as tile
from concourse import mybir
from concourse.bass_utils import run_bass_kernel_spmd
from concourse.masks import make_identity

dt = mybir.dt
AF = mybir.ActivationFunctionType
ALU = mybir.AluOpType
AX = mybir.AxisListType

N_CORES = 8
S, D, H, E = 4096, 512, 8, 64
F = 4 * D          # 2048
R = S // N_CORES   # 512 rows per core
EPS = 1e-5
SCALE = 1.0 / float(np.sqrt(E))
INV_SD = 1.0 / float(S * D)


def split_waits(nc):
    """Walrus codegen allows only one sync-wait per HW instruction. Move
    extra waits onto single-wait NoOps inserted before, same engine queue."""
    import bass_rust

    n = 0
    for bb in nc.m.functions[0].blocks:
        new_list = []
        changed = False
        for ins in bb.instructions:
            si = ins.sync_info
            if si is not None and si.on_wait is not None and len(si.on_wait) > 1:
                waits = list(si.on_wait)
                for w in waits[:-1]:
                    nop = bass_rust.InstNoOp(name=f"I-xwait-{n}")
                    n += 1
                    nop.engine = ins.engine
                    nop.sync_info = bass_rust.SyncInfo(on_wait=[w], on_update=[])
                    nc.register_instruction(nop)
                    new_list.append(nop)
                si.on_wait = waits[-1:]
                ins.sync_info = si
                changed = True
            new_list.append(ins)
        if changed:
            bb.instructions = new_list
    return nc


def build_nc():
    import contextlib

    nc = bass.Bass("TRN2", debug=False, num_devices=N_CORES)
    f32, f32r = dt.float32, dt.float32r

    # ---- I/O ----------------------------------------------------------
    x_d = nc.dram_tensor("x", [S, D], f32, kind="ExternalInput").ap()
    Wq_d = nc.dram_tensor("Wq", [H, D, E], f32, kind="ExternalInput").ap()
    Wk_d = nc.dram_tensor("Wk", [H, D, E], f32, kind="ExternalInput").ap()
    Wv_d = nc.dram_tensor("Wv", [H, D, E], f32, kind="ExternalInput").ap()
    bq_d = nc.dram_tensor("bq", [H, E], f32, kind="ExternalInput").ap()
    bk_d = nc.dram_tensor("bk", [H, E], f32, kind="ExternalInput").ap()
    bv_d = nc.dram_tensor("bv", [H, E], f32, kind="ExternalInput").ap()
    Wo_d = nc.dram_tensor("Wo", [D, D], f32, kind="ExternalInput").ap()
    bo_d = nc.dram_tensor("bo", [D], f32, kind="ExternalInput").ap()
    W1_d = nc.dram_tensor("W1", [D, F], f32, kind="ExternalInput").ap()
    b1_d = nc.dram_tensor("b1", [F], f32, kind="ExternalInput").ap()
    W2_d = nc.dram_tensor("W2", [F, D], f32, kind="ExternalInput").ap()
    b2_d = nc.dram_tensor("b2", [D], f32, kind="ExternalInput").ap()
    xr_d = nc.dram_tensor("x_rows", [R, D], f32, kind="ExternalInput").ap()
    lng_d = nc.dram_tensor("ln_g_rows", [R, D], f32, kind="ExternalInput").ap()
    lnb_d = nc.dram_tensor("ln_b_rows", [R, D], f32, kind="ExternalInput").ap()

    fin_d = nc.dram_tensor("final_rows", [R, D], f32, kind="ExternalOutput").ap()
    kp_d = nc.dram_tensor("Kp_rows", [R, D], f32, kind="ExternalOutput").ap()
    vp_d = nc.dram_tensor("Vp_rows", [R, D], f32, kind="ExternalOutput").ap()

    # row index q = qc*128 + p everywhere
    xr_v = xr_d.rearrange("(c p) d -> p c d", p=128)
    lng_v = lng_d.rearrange("(c p) d -> p c d", p=128)
    lnb_v = lnb_d.rearrange("(c p) d -> p c d", p=128)
    fin_v = fin_d.rearrange("(c p) d -> p c d", p=128)
    kp_v = kp_d.rearrange("(c p) d -> p c d", p=128)
    vp_v = vp_d.rearrange("(c p) d -> p c d", p=128)

    with tile.TileContext(nc) as tc, contextlib.ExitStack() as ctx, \
            nc.allow_low_precision(reason="bf16 matmul operands, fp32 accumulate"):
        ep = ctx.enter_context
        bf16 = dt.bfloat16

        # ---- pools ----------------------------------------------------
        single = ep(tc.tile_pool(name="single", bufs=1))
        a8 = ep(tc.tile_pool(name="a8", bufs=2))        # xa / xrT / sq
        big8 = ep(tc.tile_pool(name="big8", bufs=4))    # xt -> W1
        c8x = ep(tc.tile_pool(name="c8x", bufs=5))      # Wq/Wk/Wv -> W2
        d16 = ep(tc.tile_pool(name="d16", bufs=2))      # KTo/VTo -> h1T
        c8 = ep(tc.tile_pool(name="c8", bufs=2))        # xro(z), out1(w), out1T, fin
        qt_p = ep(tc.tile_pool(name="qt", bufs=1))      # Q^T [64, 8, R]
        ot_p = ep(tc.tile_pool(name="ot", bufs=1))      # outH^T [64, 8, R]
        evac = ep(tc.tile_pool(name="evac", bufs=4))
        pexp_p = ep(tc.tile_pool(name="pexp", bufs=3))
        kts_p = ep(tc.tile_pool(name="kts", bufs=6))
        vps_p = ep(tc.tile_pool(name="vps", bufs=3))
        otr_p = ep(tc.tile_pool(name="otr", bufs=2))
        ln_p = ep(tc.tile_pool(name="ln", bufs=2))
        wk = ep(tc.tile_pool(name="wk", bufs=2))
        sq_p = ep(tc.tile_pool(name="sq", bufs=1))
        # psum: tag "mm" 2x2banks + tag "po" 4x1bank = 8 banks
        ps_mm = ep(tc.tile_pool(name="ps_mm", bufs=2, space="PSUM"))
        ps_po = ep(tc.tile_pool(name="ps_po", bufs=4, space="PSUM"))
        dram = ep(tc.tile_pool(name="dram", bufs=1, space="DRAM"))

        # DRAM scratch for K^T and V' (streamed back during attention)
        KT_dram = dram.tile([H, 64, S], bf16)             # [h, e, t]
        VP_dram = dram.tile([32, 128, H, E + 1], bf16)    # [chunk, t%128, h, e']

        # ---- constants / small loads ---------------------------------
        ident = single.tile([128, 128], f32)
        make_identity(nc, ident[:])
        onesP = single.tile([128, 8], f32)
        nc.vector.memset(onesP[:], 1.0)
        ones1 = single.tile([1, 128], f32)
        nc.vector.memset(ones1[:], 1.0)
        ones_row = single.tile([1, 128], bf16)
        nc.vector.tensor_copy(ones_row[:], ones1[:])
        ones_row_r = single.tile([1, 128], f32r)
        nc.vector.tensor_copy(ones_row_r[:], ones1[:])
        ones8 = single.tile([128, 8], bf16)
        nc.vector.tensor_copy(ones8[:], onesP[:])

        # per-head bias at partitions 0..63 (Q^T path): [64 e, 8 h]
        bqs = single.tile([64, H], f32)
        nc.sync.dma_start(bqs[:], bq_d.rearrange("h e -> e h"))
        # packed-pair biases [(h%2)*64+e, h//2] for packed evacuations
        bks2 = single.tile([128, 4], f32)
        nc.sync.dma_start(bks2[:], bk_d.rearrange("(c h2) e -> (h2 e) c", h2=2))
        bvs2 = single.tile([128, 4], f32)
        nc.sync.dma_start(bvs2[:], bv_d.rearrange("(c h2) e -> (h2 e) c", h2=2))
        b1s = single.tile([128, 16], f32)
        nc.sync.dma_start(b1s[:], b1_d.rearrange("(c p) -> p c", p=128))
        bo_r = single.tile([1, D], bf16)
        b2_r = single.tile([1, D], bf16)
        nc.gpsimd.dma_start(bo_r[:], bo_d.rearrange("(o d) -> o d", o=1))
        nc.gpsimd.dma_start(b2_r[:], b2_d.rearrange("(o d) -> o d", o=1))
        bv_bc = single.tile([128, D], f32)
        bv_flat = bv_d.rearrange("h e -> (h e)")
        nc.gpsimd.dma_start(
            bv_bc[:],
            bass.AP(tensor=bv_flat.tensor, offset=bv_flat.offset,
                    ap=[[0, 128]] + [list(a) for a in bv_flat.ap]),
        )
        eps_t = single.tile([1, 1], f32)
        nc.vector.memset(eps_t[:], EPS)

        # Wo in per-head-row layout: [64 e, 8 h, 512 dm]
        Wo_s = single.tile([64, H, D], bf16)
        nc.gpsimd.dma_start(Wo_s[:], Wo_d.rearrange("(h e) d -> e h d", e=E))
        # Wo packed by head pair: [p = (h%2)*64+e, h//2, dm]
        Wo_p = single.tile([128, 4, D], bf16)
        nc.gpsimd.dma_start(Wo_p[:], Wo_d.rearrange("(c h2 e) d -> (h2 e) c d", h2=2, e=E))

        # Wq/Wk/Wv as [p=d%128, dc, he] with he = (h//2)*128 + (h%2)*64 + e
        w_qkv = {}
        for name, wd in (("q", Wq_d), ("k", Wk_d), ("v", Wv_d)):
            t = c8x.tile([128, 4, D], bf16, tag="c8x")
            wv4 = wd.rearrange("h (dc p) e -> dc p h e", p=128)
            for dc in range(4):
                nc.gpsimd.dma_start(
                    t[:, dc, :].rearrange("p (h e) -> p h e", e=E), wv4[dc]
                )
            w_qkv[name] = t

        QT = qt_p.tile([64, H, R], bf16)            # Q^T + bq, per head
        xro = c8.tile([128, 4, D], f32, tag="c8")   # x own rows; becomes z
        nc.sync.dma_start(xro[:], xr_v)

        # ---- phase 1: x^T tiles -> K^T, V' (to DRAM scratch) ---------
        x_v = x_d.rearrange("(tt tb p) d -> tt p tb d", p=128, tb=4)
        for tt in range(8):
            xa = a8.tile([128, 4, D], f32, tag="a8")
            nc.sync.dma_start(xa[:], x_v[tt])
            xt = big8.tile([128, 4, 512], bf16, tag="big8")  # [d%128, dc, tl]
            for dc in range(4):
                for tb in range(4):
                    ptr = ps_po.tile([128, 128], f32, tag="po")
                    nc.tensor.transpose(
                        ptr[:], xa[:, tb, dc * 128:(dc + 1) * 128], ident[:]
                    )
                    nc.vector.tensor_copy(
                        xt[:, dc, tb * 128:(tb + 1) * 128], ptr[:]
                    )
            for mc in range(4):
                pk = ps_mm.tile([128, 512], f32, tag="mm")
                for dc in range(4):
                    nc.tensor.matmul(
                        pk[:],
                        lhsT=w_qkv["k"][:, dc, mc * 128:(mc + 1) * 128],
                        rhs=xt[:, dc, :],
                        start=(dc == 0), stop=(dc == 3),
                    )
                ke = evac.tile([128, 512], bf16, tag="evac")
                nc.scalar.activation(
                    ke[:], pk[:], AF.Identity, bias=bks2[:, mc:mc + 1]
                )
                nc.sync.dma_start(
                    KT_dram[2 * mc, :, tt * 512:(tt + 1) * 512], ke[0:64, :]
                )
                nc.sync.dma_start(
                    KT_dram[2 * mc + 1, :, tt * 512:(tt + 1) * 512], ke[64:128, :]
                )
            for vc in range(4):
                pv = ps_mm.tile([128, 512], f32, tag="mm")
                for dc in range(4):
                    nc.tensor.matmul(
                        pv[:],
                        lhsT=xt[:, dc, vc * 128:(vc + 1) * 128],
                        rhs=w_qkv["v"][:, dc, :],
                        start=(dc == 0), stop=(dc == 3),
                    )
                ve = evac.tile([128, H, E + 1], bf16, tag="evac")
                nc.vector.tensor_tensor(
                    ve[:, :, 0:E],
                    pv[:].rearrange("p (h e) -> p h e", e=E),
                    bv_bc[:].rearrange("p (h e) -> p h e", e=E),
                    ALU.add,
                )
                nc.vector.tensor_copy(ve[:, :, E], ones8[:])
                nc.sync.dma_start(VP_dram[tt * 4 + vc], ve[:])

        # ---- own-rows x^T, then per-head Q^T ------------------------
        xrT = a8.tile([128, 4, R], bf16, tag="a8")
        for dc in range(4):
            for qc in range(4):
                ptr = ps_po.tile([128, 128], f32, tag="po")
                nc.tensor.transpose(
                    ptr[:], xro[:, qc, dc * 128:(dc + 1) * 128], ident[:]
                )
                nc.vector.tensor_copy(xrT[:, dc, qc * 128:(qc + 1) * 128], ptr[:])

        def own_proj_perhead(dst, w_t, bias_t):
            """dst[64, h, R] = (x_rows @ W[h])^T + b[h], per head."""
            for h in range(H):
                he_local = (h // 2) * 128 + (h % 2) * 64
                pq = ps_mm.tile([64, 512], f32, tag="mm")
                for dc in range(4):
                    nc.tensor.matmul(
                        pq[:],
                        lhsT=w_t[:, dc, he_local:he_local + 64],
                        rhs=xrT[:, dc, :],
                        start=(dc == 0), stop=(dc == 3),
                    )
                nc.scalar.activation(
                    dst[:, h, :], pq[:], AF.Identity, bias=bias_t[:, h:h + 1]
                )

        def own_proj_packed(dst, w_t, bias2_t):
            """dst[128, mc, R] = pair-packed (x_rows @ W)^T + b."""
            for mc in range(4):
                pq = ps_mm.tile([128, 512], f32, tag="mm")
                for dc in range(4):
                    nc.tensor.matmul(
                        pq[:],
                        lhsT=w_t[:, dc, mc * 128:(mc + 1) * 128],
                        rhs=xrT[:, dc, :],
                        start=(dc == 0), stop=(dc == 3),
                    )
                nc.scalar.activation(
                    dst[:, mc, :], pq[:], AF.Identity, bias=bias2_t[:, mc:mc + 1]
                )

        def wo_project_packed(src_T, out_view):
            """out_view rows = concat_h(src) @ Wo + bo (src packed [128,4,R])."""
            for qc in range(4):
                po = ps_mm.tile([128, 512], f32, tag="mm")
                for mc in range(4):
                    nc.tensor.matmul(
                        po[:],
                        lhsT=src_T[:, mc, qc * 128:(qc + 1) * 128],
                        rhs=Wo_p[:, mc, :],
                        start=(mc == 0), stop=False,
                    )
                nc.tensor.matmul(
                    po[:], lhsT=ones_row[:], rhs=bo_r[:], start=False, stop=True
                )
                ot = evac.tile([128, 512], f32, tag="evac")
                nc.vector.tensor_copy(ot[:], po[:])
                nc.sync.dma_start(out_view[:, qc, :], ot[:])

        own_proj_perhead(QT, w_qkv["q"], bqs)

        # ---- phase 2: attention (4 passes x 2 heads, skewed AV) ------
        OT = ot_p.tile([64, H, R], bf16)   # normalized outH^T per head
        for pass_ in range(4):
            h0, h1 = 2 * pass_, 2 * pass_ + 1
            po_a = ps_po.tile([E + 1, R], f32, tag="po")
            po_b = ps_po.tile([E + 1, R], f32, tag="po")
            pend = None  # (vf, pexp, ch)
            for g in range(8):
                kt_a = kts_p.tile([64, 512], bf16, tag="kts")
                nc.sync.dma_start(kt_a[:], KT_dram[h0, :, g * 512:(g + 1) * 512])
                kt_b = kts_p.tile([64, 512], bf16, tag="kts")
                nc.sync.dma_start(kt_b[:], KT_dram[h1, :, g * 512:(g + 1) * 512])
                for cc in range(4):
                    ch = g * 4 + cc
                    vf = vps_p.tile([128, H, E + 1], bf16, tag="vps")
                    nc.sync.dma_start(vf[:], VP_dram[ch])
                    pl = ps_mm.tile([128, 2, 512], f32, tag="mm")
                    nc.tensor.matmul(
                        pl[:, 0, :],
                        lhsT=kt_a[:, cc * 128:(cc + 1) * 128],
                        rhs=QT[:, h0, :], start=True, stop=True,
                    )
                    nc.tensor.matmul(
                        pl[:, 1, :],
                        lhsT=kt_b[:, cc * 128:(cc + 1) * 128],
                        rhs=QT[:, h1, :], start=True, stop=True,
                    )
                    pexp = pexp_p.tile([128, 2, 512], bf16, tag="pexp")
                    nc.scalar.activation(pexp[:], pl[:], AF.Exp, scale=SCALE)
                    if pend is not None:
                        pvf, ppexp, pch = pend
                        nc.tensor.matmul(
                            po_a[:], lhsT=pvf[:, h0, :], rhs=ppexp[:, 0, :],
                            start=(pch == 0), stop=False,
                        )
                        nc.tensor.matmul(
                            po_b[:], lhsT=pvf[:, h1, :], rhs=ppexp[:, 1, :],
                            start=(pch == 0), stop=False,
                        )
                    pend = (vf, pexp, ch)
            pvf, ppexp, pch = pend
            nc.tensor.matmul(
                po_a[:], lhsT=pvf[:, h0, :], rhs=ppexp[:, 0, :],
                start=False, stop=True,
            )
            nc.tensor.matmul(
                po_b[:], lhsT=pvf[:, h1, :], rhs=ppexp[:, 1, :],
                start=False, stop=True,
            )
            # normalize rows 0..63 by the ones-column row 64
            for po_t, h in ((po_a, h0), (po_b, h1)):
                otr = otr_p.tile([E + 1, R], f32, tag="otr")
                nc.scalar.copy(otr[:], po_t[:])
                rden = otr_p.tile([1, R], f32r, tag="rden")
                nc.vector.reciprocal(rden[:], otr[E:E + 1, :])
                pb = ps_mm.tile([E, R], f32, tag="mm")
                nc.tensor.matmul(
                    pb[:], lhsT=ones_row_r[:, 0:E], rhs=rden[:],
                    start=True, stop=True,
                )
                nc.vector.tensor_tensor(OT[:, h, :], otr[0:E, :], pb[:], ALU.mult)

        # ---- phase 3: out proj + residual + global LN1 ---------------
        z = xro  # in place: z = x + out
        for qc in range(4):
            po = ps_mm.tile([128, 512], f32, tag="mm")
            for h in range(H):
                nc.tensor.matmul(
                    po[:],
                    lhsT=OT[:, h, qc * 128:(qc + 1) * 128],
                    rhs=Wo_s[:, h, :],
                    start=(h == 0), stop=False,
                )
            nc.tensor.matmul(
                po[:], lhsT=ones_row[:], rhs=bo_r[:], start=False, stop=True
            )
            nc.vector.tensor_tensor(z[:, qc, :], po[:], xro[:, qc, :], ALU.add)

        def stats_start(src_t, tag):
            """Partial [sum, sumsq] -> AllReduce; returns output dram tile."""
            sums = wk.tile([128, 2], f32, tag=f"sums{tag}")
            nc.vector.tensor_reduce(
                out=sums[:, 0:1], in_=src_t[:], axis=AX.XY, op=ALU.add
            )
            sq = sq_p.tile([128, 4, D], f32, tag="sq")
            nc.scalar.activation(
                sq[:], src_t[:], AF.Square, accum_out=sums[:, 1:2]
            )
            pr = ps_po.tile([1, 2], f32, tag="po")
            nc.tensor.matmul(
                pr[:], lhsT=onesP[:, 0:1], rhs=sums[:], start=True, stop=True
            )
            part = wk.tile([1, 2], f32, tag=f"part{tag}")
            nc.vector.tensor_copy(part[:], pr[:])
            cin = dram.tile([1, 2], f32)
            cout = dram.tile([1, 2], f32)
            nc.sync.dma_start(cin[:], part[:])
            nc.gpsimd.collective_compute(
                "AllReduce", ALU.add,
                replica_groups=[list(range(N_CORES))],
                ins=[cin[:]], outs=[cout[:]],
            )
            return cout

        def stats_finish(cout, tag):
            """-> [128, 2] sbuf tile: [:,0]=rstd, [:,1]=-mu*rstd (global)."""
            tot = wk.tile([1, 2], f32, tag=f"tot{tag}")
            nc.sync.dma_start(tot[:], cout[:])
            sc = wk.tile([1, 6], f32, tag=f"sc{tag}")
            mu, m2 = sc[0:1, 0:1], sc[0:1, 1:2]
            nc.vector.tensor_scalar_mul(mu, tot[0:1, 0:1], INV_SD)
            nc.vector.tensor_scalar_mul(m2, tot[0:1, 1:2], INV_SD)
            nc.vector.tensor_tensor(sc[0:1, 2:3], mu, mu, ALU.mult)
            nc.vector.tensor_tensor(sc[0:1, 3:4], m2, sc[0:1, 2:3], ALU.subtract)
            nc.scalar.activation(sc[0:1, 4:5], sc[0:1, 3:4], AF.Sqrt, bias=eps_t[:])
            st2 = wk.tile([1, 2], f32r, tag=f"st2{tag}")
            nc.vector.reciprocal(st2[0:1, 0:1], sc[0:1, 4:5])        # rstd
            nc.vector.tensor_tensor(sc[0:1, 5:6], mu, st2[0:1, 0:1], ALU.mult)
            nc.vector.tensor_scalar_mul(st2[0:1, 1:2], sc[0:1, 5:6], -1.0)
            pbc = ps_po.tile([128, 2], f32, tag="po")
            nc.tensor.matmul(pbc[:], lhsT=ones_row_r[:], rhs=st2[:],
                             start=True, stop=True)
            stb = wk.tile([128, 2], f32, tag=f"stb{tag}")
            nc.vector.tensor_copy(stb[:], pbc[:])
            return stb

        def ln_apply(dst_tile, src_t, stb, store_view=None):
            for qc in range(4):
                g_t = ln_p.tile([128, D], f32, tag="g")
                b_t = ln_p.tile([128, D], f32, tag="b")
                nc.sync.dma_start(g_t[:], lng_v[:, qc, :])
                nc.sync.dma_start(b_t[:], lnb_v[:, qc, :])
                n_t = evac.tile([128, D], f32, tag="evac")
                nc.scalar.activation(
                    n_t[:], src_t[:, qc, :], AF.Identity,
                    bias=stb[:, 1:2], scale=stb[:, 0:1],
                )
                nc.vector.tensor_tensor(n_t[:], n_t[:], g_t[:], ALU.mult)
                nc.vector.tensor_tensor(dst_tile[:, qc, :], n_t[:], b_t[:], ALU.add)
                if store_view is not None:
                    nc.sync.dma_start(store_view[:, qc, :], dst_tile[:, qc, :])

        cout1 = stats_start(z, "a")
        # Kp fills the first AllReduce's latency window
        KTo = d16.tile([128, 4, R], bf16, tag="d16")
        own_proj_packed(KTo, w_qkv["k"], bks2)
        wo_project_packed(KTo, kp_v)
        stb1 = stats_finish(cout1, "a")
        out1 = c8.tile([128, 4, D], f32, tag="c8")
        ln_apply(out1, z, stb1)
        out1T = c8.tile([128, 4, R], bf16, tag="c8")
        for dc in range(4):
            for qc in range(4):
                ptr = ps_po.tile([128, 128], f32, tag="po")
                nc.tensor.transpose(
                    ptr[:], out1[:, qc, dc * 128:(dc + 1) * 128], ident[:]
                )
                nc.vector.tensor_copy(out1T[:, dc, qc * 128:(qc + 1) * 128], ptr[:])

        # ---- phase 4: MLP + residual + global LN2 --------------------
        W1_v = W1_d.rearrange("(dc p) f -> dc p f", p=128)
        W1_s = []
        for j in range(4):
            t = big8.tile([128, F], bf16, tag="big8")
            nc.gpsimd.dma_start(t[:], W1_v[j])
            W1_s.append(t)
        W2_v = W2_d.rearrange("(g fc p) d -> g p fc d", p=128, fc=4)
        W2_s = []
        for j in range(4):
            t = c8x.tile([128, 4, D], bf16, tag="c8x")
            nc.gpsimd.dma_start(t[:], W2_v[j])
            W2_s.append(t)
        h1T = []
        for j in range(2):
            h1t_half = d16.tile([128, 8, R], bf16, tag="d16")
            h1T.append(h1t_half)
        for fm in range(16):
            ph = ps_mm.tile([128, R], f32, tag="mm")
            for dc in range(4):
                nc.tensor.matmul(
                    ph[:],
                    lhsT=W1_s[dc][:, fm * 128:(fm + 1) * 128],
                    rhs=out1T[:, dc, :],
                    start=(dc == 0), stop=(dc == 3),
                )
            nc.scalar.activation(
                h1T[fm // 8][:, fm % 8, :], ph[:], AF.Relu, bias=b1s[:, fm:fm + 1]
            )
        w = out1  # in place: w = out1 + out2
        for qc in range(4):
            po = ps_mm.tile([128, D], f32, tag="mm")
            for fm in range(16):
                nc.tensor.matmul(
                    po[:],
                    lhsT=h1T[fm // 8][:, fm % 8, qc * 128:(qc + 1) * 128],
                    rhs=W2_s[fm // 4][:, fm % 4, :],
                    start=(fm == 0), stop=False,
                )
            nc.tensor.matmul(
                po[:], lhsT=ones_row[:], rhs=b2_r[:], start=False, stop=True
            )
            nc.vector.tensor_tensor(w[:, qc, :], po[:], out1[:, qc, :], ALU.add)

        cout2 = stats_start(w, "b")
        # Vp fills the second AllReduce's latency window
        VTo = d16.tile([128, 4, R], bf16, tag="d16")
        own_proj_packed(VTo, w_qkv["v"], bvs2)
        wo_project_packed(VTo, vp_v)
        stb2 = stats_finish(cout2, "b")
        fin_s = c8.tile([128, 4, D], f32, tag="c8")
        ln_apply(fin_s, w, stb2, store_view=fin_v)

    split_waits(nc)
    return nc


_NC_CACHE = None


def _get_nc():
    global _NC_CACHE
    if _NC_CACHE is None:
        _NC_CACHE = build_nc()
    return _NC_CACHE


def kernel(**inputs):
    inp = {k: np.ascontiguousarray(np.asarray(v, dtype=np.float32))
           for k, v in inputs.items()}
    in_maps = []
    for c in range(N_CORES):
        rows = slice(c * R, (c + 1) * R)
        in_maps.append(dict(
            x=inp["x"], Wq=inp["Wq"], Wk=inp["Wk"], Wv=inp["Wv"],
            bq=inp["bq"], bk=inp["bk"], bv=inp["bv"],
            Wo=inp["Wo"], bo=inp["bo"], W1=inp["W1"], b1=inp["b1"],
            W2=inp["W2"], b2=inp["b2"],
            x_rows=inp["x"][rows],
            ln_g_rows=inp["ln_g"][rows], ln_b_rows=inp["ln_b"][rows],
        ))
    nc = _get_nc()
    res = run_bass_kernel_spmd(nc, in_maps, list(range(N_CORES)))
    final = np.concatenate([res.results[c]["final_rows"] for c in range(N_CORES)])
    Kp = np.concatenate([res.results[c]["Kp_rows"] for c in range(N_CORES)])
    Vp = np.concatenate([res.results[c]["Vp_rows"] for c in range(N_CORES)])
    return (final, Kp, Vp)


# revision 26
# speedup vs baseline: 1.4624x; 1.1434x over previous
"""Trainium2 Bass kernel for nn_Encoder (S=4096, D=512, H=8, E=64).

Sharding: sequence-parallel over 8 cores. Each core computes the full K/V
(every query needs them) plus attention/MLP for its own 512 rows; the only
cross-core traffic is two 8-byte AllReduces for the global LayerNorm
statistics (the reference normalizes jointly over the whole [S, D] tensor).
The host concatenates the per-core row shards.

Per-core dataflow:
  - x^T tiles built with PE transposes; K^T [he, t] and V [t, he] follow as
    fp32r matmuls (two heads packed per 128-wide stationary), written to a
    DRAM scratch and streamed back during attention (SBUF can't hold both).
  - logits are computed transposed, L^T[t, q] = K^T-slice.T @ Q^T, so the
    Exp output is already the A@V moving operand; softmax denominators fall
    out of a ones-column appended to V (row 64 of the accumulator).
  - per-head tensors (Q^T, outH^T, own K^T/V^T) live at partitions 0..63
    with the head index on a free dim, so every matmul/DVE op sees matching
    base partitions.
  - the MLP uses h1^T = W1-slice.T @ out1^T so no intermediate needs an
    explicit transpose.
"""

import os

os.environ.setdefault("JAX_PLATFORMS", "axon")

import numpy as np

import concourse.bass as bass
import concourse.tile as tile
from concourse import mybir
from concourse.bass_utils import run_bass_kernel_spmd
from concourse.masks import make_identity

dt = mybir.dt
AF = mybir.ActivationFunctionType
ALU = mybir.AluOpType
AX = mybir.AxisListType

N_CORES = 8
S, D, H, E = 4096, 512, 8, 64
F = 4 * D          # 2048
R = S // N_CORES   # 512 rows per core
EPS = 1e-5
SCALE = 1.0 / float(np.sqrt(E))
INV_SD = 1.0 / float(S * D)


def split_waits(nc):
    """Walrus codegen allows only one sync-wait per HW instruction. Move
    extra waits onto single-wait NoOps inserted before, same engine queue."""
    import bass_rust

    n = 0
    for bb in nc.m.functions[0].blocks:
        new_list = []
        changed = False
        for ins in bb.instructions:
            si = ins.sync_info
            if si is not None and si.on_wait is not None and len(si.on_wait) > 1:
                waits = list(si.on_wait)
                for w in waits[:-1]:
                    nop = bass_rust.InstNoOp(name=f"I-xwait-{n}")
                    n += 1
                    nop.engine = ins.engine
                    nop.sync_info = bass_rust.SyncInfo(on_wait=[w], on_update=[])
                    nc.register_instruction(nop)
                    new_list.append(nop)
                si.on_wait = waits[-1:]
                ins.sync_info = si
                changed = True
            new_list.append(ins)
        if changed:
            bb.instructions = new_list
    return nc


def build_nc():
    import contextlib

    nc = bass.Bass("TRN2", debug=False, num_devices=N_CORES)
    f32, f32r = dt.float32, dt.float32r

    # ---- I/O ----------------------------------------------------------
    x_d = nc.dram_tensor("x", [S, D], f32, kind="ExternalInput").ap()
    Wq_d = nc.dram_tensor("Wq", [H, D, E], f32, kind="ExternalInput").ap()
    Wk_d = nc.dram_tensor("Wk", [H, D, E], f32, kind="ExternalInput").ap()
    Wv_d = nc.dram_tensor("Wv", [H, D, E], f32, kind="ExternalInput").ap()
    bq_d = nc.dram_tensor("bq", [H, E], f32, kind="ExternalInput").ap()
    bk_d = nc.dram_tensor("bk", [H, E], f32, kind="ExternalInput").ap()
    bv_d = nc.dram_tensor("bv", [H, E], f32, kind="ExternalInput").ap()
    Wo_d = nc.dram_tensor("Wo", [D, D], f32, kind="ExternalInput").ap()
    bo_d = nc.dram_tensor("bo", [D], f32, kind="ExternalInput").ap()
    W1_d = nc.dram_tensor("W1", [D, F], f32, kind="ExternalInput").ap()
    b1_d = nc.dram_tensor("b1", [F], f32, kind="ExternalInput").ap()
    W2_d = nc.dram_tensor("W2", [F, D], f32, kind="ExternalInput").ap()
    b2_d = nc.dram_tensor("b2", [D], f32, kind="ExternalInput").ap()
    xr_d = nc.dram_tensor("x_rows", [R, D], f32, kind="ExternalInput").ap()
    lng_d = nc.dram_tensor("ln_g_rows", [R, D], f32, kind="ExternalInput").ap()
    lnb_d = nc.dram_tensor("ln_b_rows", [R, D], f32, kind="ExternalInput").ap()

    fin_d = nc.dram_tensor("final_rows", [R, D], f32, kind="ExternalOutput").ap()
    kp_d = nc.dram_tensor("Kp_rows", [R, D], f32, kind="ExternalOutput").ap()
    vp_d = nc.dram_tensor("Vp_rows", [R, D], f32, kind="ExternalOutput").ap()

    # row index q = qc*128 + p everywhere
    xr_v = xr_d.rearrange("(c p) d -> p c d", p=128)
    lng_v = lng_d.rearrange("(c p) d -> p c d", p=128)
    lnb_v = lnb_d.rearrange("(c p) d -> p c d", p=128)
    fin_v = fin_d.rearrange("(c p) d -> p c d", p=128)
    kp_v = kp_d.rearrange("(c p) d -> p c d", p=128)
    vp_v = vp_d.rearrange("(c p) d -> p c d", p=128)

    with tile.TileContext(nc) as tc, contextlib.ExitStack() as ctx, \
            nc.allow_low_precision(reason="bf16 matmul operands, fp32 accumulate"):
        ep = ctx.enter_context
        bf16 = dt.bfloat16

        # ---- pools ----------------------------------------------------
        single = ep(tc.tile_pool(name="single", bufs=1))
        a8 = ep(tc.tile_pool(name="a8", bufs=2))        # xa / xrT / sq
        big8 = ep(tc.tile_pool(name="big8", bufs=4))    # xt -> W1
        c8x = ep(tc.tile_pool(name="c8x", bufs=5))      # Wq/Wk/Wv -> W2
        d16 = ep(tc.tile_pool(name="d16", bufs=2))      # KTo/VTo -> h1T
        c8 = ep(tc.tile_pool(name="c8", bufs=2))        # xro(z), out1(w), out1T, fin
        qt_p = ep(tc.tile_pool(name="qt", bufs=1))      # Q^T [64, 8, R]
        ot_p = ep(tc.tile_pool(name="ot", bufs=1))      # outH^T [64, 8, R]
        evac = ep(tc.tile_pool(name="evac", bufs=4))
        pexp_p = ep(tc.tile_pool(name="pexp", bufs=3))
        vps_p = ep(tc.tile_pool(name="vps", bufs=3))
        otr_p = ep(tc.tile_pool(name="otr", bufs=2))
        ln_p = ep(tc.tile_pool(name="ln", bufs=2))
        wk = ep(tc.tile_pool(name="wk", bufs=2))
        sq_p = ep(tc.tile_pool(name="sq", bufs=1))
        # psum: tag "mm" 2x2banks + tag "po" 4x1bank = 8 banks
        ps_mm = ep(tc.tile_pool(name="ps_mm", bufs=2, space="PSUM"))
        ps_po = ep(tc.tile_pool(name="ps_po", bufs=4, space="PSUM"))
        dram = ep(tc.tile_pool(name="dram", bufs=1, space="DRAM"))

        # DRAM scratch for K^T and V' (streamed back during attention)
        KT_dram = dram.tile([H, 64, S], bf16)             # [h, e, t]
        VP_dram = dram.tile([32, 128, H, E + 1], bf16)    # [chunk, t%128, h, e']

        # ---- constants / small loads ---------------------------------
        ident = single.tile([128, 128], f32)
        make_identity(nc, ident[:])
        onesP = single.tile([128, 8], f32)
        nc.vector.memset(onesP[:], 1.0)
        ones1 = single.tile([1, 128], f32)
        nc.vector.memset(ones1[:], 1.0)
        ones_row = single.tile([1, 128], bf16)
        nc.vector.tensor_copy(ones_row[:], ones1[:])
        ones_row_r = single.tile([1, 128], f32r)
        nc.vector.tensor_copy(ones_row_r[:], ones1[:])
        ones8 = single.tile([128, 8], bf16)
        nc.vector.tensor_copy(ones8[:], onesP[:])

        # per-head bias at partitions 0..63 (Q^T path): [64 e, 8 h]
        bqs = single.tile([64, H], f32)
        nc.sync.dma_start(bqs[:], bq_d.rearrange("h e -> e h"))
        # packed-pair biases [(h%2)*64+e, h//2] for packed evacuations
        bks2 = single.tile([128, 4], f32)
        nc.sync.dma_start(bks2[:], bk_d.rearrange("(c h2) e -> (h2 e) c", h2=2))
        bvs2 = single.tile([128, 4], f32)
        nc.sync.dma_start(bvs2[:], bv_d.rearrange("(c h2) e -> (h2 e) c", h2=2))
        b1s = single.tile([128, 16], f32)
        nc.sync.dma_start(b1s[:], b1_d.rearrange("(c p) -> p c", p=128))
        bo_r = single.tile([1, D], bf16)
        b2_r = single.tile([1, D], bf16)
        nc.gpsimd.dma_start(bo_r[:], bo_d.rearrange("(o d) -> o d", o=1))
        nc.gpsimd.dma_start(b2_r[:], b2_d.rearrange("(o d) -> o d", o=1))
        bv_bc = single.tile([128, D], f32)
        bv_flat = bv_d.rearrange("h e -> (h e)")
        nc.gpsimd.dma_start(
            bv_bc[:],
            bass.AP(tensor=bv_flat.tensor, offset=bv_flat.offset,
                    ap=[[0, 128]] + [list(a) for a in bv_flat.ap]),
        )
        eps_t = single.tile([1, 1], f32)
        nc.vector.memset(eps_t[:], EPS)

        # Wo in per-head-row layout padded to 128 rows (bottom zeroed so a
        # K=128 contraction against zero-padded outH^T is exact)
        Wo_s = single.tile([128, H, D], bf16)
        nc.vector.memset(Wo_s[:], 0.0)
        nc.gpsimd.dma_start(Wo_s[0:64, :, :], Wo_d.rearrange("(h e) d -> e h d", e=E))
        # Wo packed by head pair: [p = (h%2)*64+e, h//2, dm]
        Wo_p = single.tile([128, 4, D], bf16)
        nc.gpsimd.dma_start(Wo_p[:], Wo_d.rearrange("(c h2 e) d -> (h2 e) c d", h2=2, e=E))

        # Wq/Wk/Wv as [p=d%128, dc, he] with he = (h//2)*128 + (h%2)*64 + e
        w_qkv = {}
        for name, wd in (("q", Wq_d), ("k", Wk_d), ("v", Wv_d)):
            t = c8x.tile([128, 4, D], bf16, tag="c8x")
            wv4 = wd.rearrange("h (dc p) e -> dc p h e", p=128)
            for dc in range(4):
                nc.gpsimd.dma_start(
                    t[:, dc, :].rearrange("p (h e) -> p h e", e=E), wv4[dc]
                )
            w_qkv[name] = t

        QT = qt_p.tile([128, H, R], bf16)           # Q^T + bq, zero-padded rows
        nc.vector.memset(QT[64:128, :, :], 0.0)
        kt_ring = []
        for j in range(6):
            kt_t = single.tile([128, 512], bf16, name=f"ktr{j}")
            nc.vector.memset(kt_t[:], 0.0)
            kt_ring.append(kt_t)
        xro = c8.tile([128, 4, D], f32, tag="c8")   # x own rows; becomes z
        nc.sync.dma_start(xro[:], xr_v)

        # ---- phase 1: x^T tiles -> K^T, V' (to DRAM scratch) ---------
        x_v = x_d.rearrange("(tt tb p) d -> tt p tb d", p=128, tb=4)
        for tt in range(8):
            xa = a8.tile([128, 4, D], f32, tag="a8")
            nc.sync.dma_start(xa[:], x_v[tt])
            xt = big8.tile([128, 4, 512], bf16, tag="big8")  # [d%128, dc, tl]
            for dc in range(4):
                for tb in range(4):
                    ptr = ps_po.tile([128, 128], f32, tag="po")
                    nc.tensor.transpose(
                        ptr[:], xa[:, tb, dc * 128:(dc + 1) * 128], ident[:]
                    )
                    if (dc + tb) % 2 == 0:
                        nc.vector.tensor_copy(
                            xt[:, dc, tb * 128:(tb + 1) * 128], ptr[:]
                        )
                    else:
                        nc.scalar.copy(
                            xt[:, dc, tb * 128:(tb + 1) * 128], ptr[:]
                        )
            for mc in range(4):
                pk = ps_mm.tile([128, 512], f32, tag="mm")
                for dc in range(4):
                    nc.tensor.matmul(
                        pk[:],
                        lhsT=w_qkv["k"][:, dc, mc * 128:(mc + 1) * 128],
                        rhs=xt[:, dc, :],
                        start=(dc == 0), stop=(dc == 3),
                    )
                ke = evac.tile([128, 512], bf16, tag="evac")
                nc.scalar.activation(
                    ke[:], pk[:], AF.Identity, bias=bks2[:, mc:mc + 1]
                )
                nc.sync.dma_start(
                    KT_dram[2 * mc, :, tt * 512:(tt + 1) * 512], ke[0:64, :]
                )
                nc.sync.dma_start(
                    KT_dram[2 * mc + 1, :, tt * 512:(tt + 1) * 512], ke[64:128, :]
                )
            for vc in range(4):
                pv = ps_mm.tile([128, 512], f32, tag="mm")
                for dc in range(4):
                    nc.tensor.matmul(
                        pv[:],
                        lhsT=xt[:, dc, vc * 128:(vc + 1) * 128],
                        rhs=w_qkv["v"][:, dc, :],
                        start=(dc == 0), stop=(dc == 3),
                    )
                ve = evac.tile([128, H, E + 1], bf16, tag="evac")
                nc.vector.tensor_tensor(
                    ve[:, :, 0:E],
                    pv[:].rearrange("p (h e) -> p h e", e=E),
                    bv_bc[:].rearrange("p (h e) -> p h e", e=E),
                    ALU.add,
                )
                nc.vector.tensor_copy(ve[:, :, E], ones8[:])
                nc.sync.dma_start(VP_dram[tt * 4 + vc], ve[:])

        # ---- own-rows x^T, then per-head Q^T ------------------------
        xrT = a8.tile([128, 4, R], bf16, tag="a8")
        for dc in range(4):
            for qc in range(4):
                ptr = ps_po.tile([128, 128], f32, tag="po")
                nc.tensor.transpose(
                    ptr[:], xro[:, qc, dc * 128:(dc + 1) * 128], ident[:]
                )
                nc.vector.tensor_copy(xrT[:, dc, qc * 128:(qc + 1) * 128], ptr[:])

        def own_proj_perhead(dst, w_t, bias_t):
            """dst[64, h, R] = (x_rows @ W[h])^T + b[h], per head."""
            for h in range(H):
                he_local = (h // 2) * 128 + (h % 2) * 64
                pq = ps_mm.tile([64, 512], f32, tag="mm")
                for dc in range(4):
                    nc.tensor.matmul(
                        pq[:],
                        lhsT=w_t[:, dc, he_local:he_local + 64],
                        rhs=xrT[:, dc, :],
                        start=(dc == 0), stop=(dc == 3),
                    )
                nc.scalar.activation(
                    dst[0:64, h, :], pq[:], AF.Identity, bias=bias_t[:, h:h + 1]
                )

        def own_proj_packed(dst, w_t, bias2_t):
            """dst[128, mc, R] = pair-packed (x_rows @ W)^T + b."""
            for mc in range(4):
                pq = ps_mm.tile([128, 512], f32, tag="mm")
                for dc in range(4):
                    nc.tensor.matmul(
                        pq[:],
                        lhsT=w_t[:, dc, mc * 128:(mc + 1) * 128],
                        rhs=xrT[:, dc, :],
                        start=(dc == 0), stop=(dc == 3),
                    )
                nc.scalar.activation(
                    dst[:, mc, :], pq[:], AF.Identity, bias=bias2_t[:, mc:mc + 1]
                )

        def wo_project_packed(src_T, out_view):
            """out_view rows = concat_h(src) @ Wo + bo (src packed [128,4,R])."""
            for qc in range(4):
                po = ps_mm.tile([128, 512], f32, tag="mm")
                for mc in range(4):
                    nc.tensor.matmul(
                        po[:],
                        lhsT=src_T[:, mc, qc * 128:(qc + 1) * 128],
                        rhs=Wo_p[:, mc, :],
                        start=(mc == 0), stop=False,
                    )
                nc.tensor.matmul(
                    po[:], lhsT=ones_row[:], rhs=bo_r[:], start=False, stop=True
                )
                ot = evac.tile([128, 512], f32, tag="evac")
                nc.vector.tensor_copy(ot[:], po[:])
                nc.sync.dma_start(out_view[:, qc, :], ot[:])

        own_proj_perhead(QT, w_qkv["q"], bqs)

        # ---- phase 2: attention (4 passes x 2 heads, skewed AV) ------
        OT = ot_p.tile([128, H, R], bf16)  # normalized outH^T, zero-padded
        nc.vector.memset(OT[64:128, :, :], 0.0)
        kt_i = 0
        for pass_ in range(4):
            h0, h1 = 2 * pass_, 2 * pass_ + 1
            po_a = ps_po.tile([E + 1, R], f32, tag="po")
            po_b = ps_po.tile([E + 1, R], f32, tag="po")
            pend = None  # (vf, pexp, ch)
            for g in range(8):
                kt_a = kt_ring[kt_i % 6]
                kt_i += 1
                nc.sync.dma_start(kt_a[0:64, :], KT_dram[h0, :, g * 512:(g + 1) * 512])
                kt_b = kt_ring[kt_i % 6]
                kt_i += 1
                nc.sync.dma_start(kt_b[0:64, :], KT_dram[h1, :, g * 512:(g + 1) * 512])
                for cc in range(4):
                    ch = g * 4 + cc
                    vf = vps_p.tile([128, H, E + 1], bf16, tag="vps")
                    nc.sync.dma_start(vf[:], VP_dram[ch])
                    pl = ps_mm.tile([128, 2, 512], f32, tag="mm")
                    nc.tensor.matmul(
                        pl[:, 0, :],
                        lhsT=kt_a[:, cc * 128:(cc + 1) * 128],
                        rhs=QT[:, h0, :], start=True, stop=True,
                    )
                    nc.tensor.matmul(
                        pl[:, 1, :],
                        lhsT=kt_b[:, cc * 128:(cc + 1) * 128],
                        rhs=QT[:, h1, :], start=True, stop=True,
                    )
                    pexp = pexp_p.tile([128, 2, 512], bf16, tag="pexp")
                    nc.scalar.activation(pexp[:], pl[:], AF.Exp, scale=SCALE)
                    if pend is not None:
                        pvf, ppexp, pch = pend
                        nc.tensor.matmul(
                            po_a[:], lhsT=pvf[:, h0, :], rhs=ppexp[:, 0, :],
                            start=(pch == 0), stop=False,
                        )
                        nc.tensor.matmul(
                            po_b[:], lhsT=pvf[:, h1, :], rhs=ppexp[:, 1, :],
                            start=(pch == 0), stop=False,
                        )
                    pend = (vf, pexp, ch)
            pvf, ppexp, pch = pend
            nc.tensor.matmul(
                po_a[:], lhsT=pvf[:, h0, :], rhs=ppexp[:, 0, :],
                start=False, stop=True,
            )
            nc.tensor.matmul(
                po_b[:], lhsT=pvf[:, h1, :], rhs=ppexp[:, 1, :],
                start=False, stop=True,
            )
            # normalize rows 0..63 by the ones-column row 64
            for po_t, h in ((po_a, h0), (po_b, h1)):
                otr = otr_p.tile([E + 1, R], f32, tag="otr")
                nc.scalar.copy(otr[:], po_t[:])
                rden = otr_p.tile([1, R], f32r, tag="rden")
                nc.vector.reciprocal(rden[:], otr[E:E + 1, :])
                pb = ps_mm.tile([E, R], f32, tag="mm")
                nc.tensor.matmul(
                    pb[:], lhsT=ones_row_r[:, 0:E], rhs=rden[:],
                    start=True, stop=True,
                )
                nc.vector.tensor_tensor(OT[0:64, h, :], otr[0:E, :], pb[:], ALU.mult)

        # ---- phase 3: out proj + residual + global LN1 ---------------
        z = xro  # in place: z = x + out
        for qc in range(4):
            po = ps_mm.tile([128, 512], f32, tag="mm")
            for h in range(H):
                nc.tensor.matmul(
                    po[:],
                    lhsT=OT[:, h, qc * 128:(qc + 1) * 128],
                    rhs=Wo_s[:, h, :],
                    start=(h == 0), stop=False,
                )
            nc.tensor.matmul(
                po[:], lhsT=ones_row[:], rhs=bo_r[:], start=False, stop=True
            )
            nc.vector.tensor_tensor(z[:, qc, :], po[:], xro[:, qc, :], ALU.add)

        def stats_start(src_t, tag):
            """Partial [sum, sumsq] -> AllReduce; returns output dram tile."""
            sums = wk.tile([128, 2], f32, tag=f"sums{tag}")
            nc.vector.tensor_reduce(
                out=sums[:, 0:1], in_=src_t[:], axis=AX.XY, op=ALU.add
            )
            sq = sq_p.tile([128, 4, D], f32, tag="sq")
            nc.scalar.activation(
                sq[:], src_t[:], AF.Square, accum_out=sums[:, 1:2]
            )
            pr = ps_po.tile([1, 2], f32, tag="po")
            nc.tensor.matmul(
                pr[:], lhsT=onesP[:, 0:1], rhs=sums[:], start=True, stop=True
            )
            part = wk.tile([1, 2], f32, tag=f"part{tag}")
            nc.vector.tensor_copy(part[:], pr[:])
            cin = dram.tile([1, 2], f32)
            cout = dram.tile([1, 2], f32)
            nc.sync.dma_start(cin[:], part[:])
            nc.gpsimd.collective_compute(
                "AllReduce", ALU.add,
                replica_groups=[list(range(N_CORES))],
                ins=[cin[:]], outs=[cout[:]],
            )
            return cout

        def stats_finish(cout, tag):
            """-> [128, 2] sbuf tile: [:,0]=rstd, [:,1]=-mu*rstd (global)."""
            tot = wk.tile([1, 2], f32, tag=f"tot{tag}")
            nc.sync.dma_start(tot[:], cout[:])
            sc = wk.tile([1, 6], f32, tag=f"sc{tag}")
            mu, m2 = sc[0:1, 0:1], sc[0:1, 1:2]
            nc.vector.tensor_scalar_mul(mu, tot[0:1, 0:1], INV_SD)
            nc.vector.tensor_scalar_mul(m2, tot[0:1, 1:2], INV_SD)
            nc.vector.tensor_tensor(sc[0:1, 2:3], mu, mu, ALU.mult)
            nc.vector.tensor_tensor(sc[0:1, 3:4], m2, sc[0:1, 2:3], ALU.subtract)
            nc.scalar.activation(sc[0:1, 4:5], sc[0:1, 3:4], AF.Sqrt, bias=eps_t[:])
            st2 = wk.tile([1, 2], f32r, tag=f"st2{tag}")
            nc.vector.reciprocal(st2[0:1, 0:1], sc[0:1, 4:5])        # rstd
            nc.vector.tensor_tensor(sc[0:1, 5:6], mu, st2[0:1, 0:1], ALU.mult)
            nc.vector.tensor_scalar_mul(st2[0:1, 1:2], sc[0:1, 5:6], -1.0)
            pbc = ps_po.tile([128, 2], f32, tag="po")
            nc.tensor.matmul(pbc[:], lhsT=ones_row_r[:], rhs=st2[:],
                             start=True, stop=True)
            stb = wk.tile([128, 2], f32, tag=f"stb{tag}")
            nc.vector.tensor_copy(stb[:], pbc[:])
            return stb

        def ln_apply(dst_tile, src_t, stb, store_view=None):
            for qc in range(4):
                g_t = ln_p.tile([128, D], f32, tag="g")
                b_t = ln_p.tile([128, D], f32, tag="b")
                nc.sync.dma_start(g_t[:], lng_v[:, qc, :])
                nc.sync.dma_start(b_t[:], lnb_v[:, qc, :])
                n_t = evac.tile([128, D], f32, tag="evac")
                nc.scalar.activation(
                    n_t[:], src_t[:, qc, :], AF.Identity,
                    bias=stb[:, 1:2], scale=stb[:, 0:1],
                )
                nc.vector.tensor_tensor(n_t[:], n_t[:], g_t[:], ALU.mult)
                nc.vector.tensor_tensor(dst_tile[:, qc, :], n_t[:], b_t[:], ALU.add)
                if store_view is not None:
                    nc.sync.dma_start(store_view[:, qc, :], dst_tile[:, qc, :])

        cout1 = stats_start(z, "a")
        # Kp fills the first AllReduce's latency window
        KTo = d16.tile([128, 4, R], bf16, tag="d16")
        own_proj_packed(KTo, w_qkv["k"], bks2)
        wo_project_packed(KTo, kp_v)
        stb1 = stats_finish(cout1, "a")
        out1 = c8.tile([128, 4, D], f32, tag="c8")
        ln_apply(out1, z, stb1)
        out1T = c8.tile([128, 4, R], bf16, tag="c8")
        for dc in range(4):
            for qc in range(4):
                ptr = ps_po.tile([128, 128], f32, tag="po")
                nc.tensor.transpose(
                    ptr[:], out1[:, qc, dc * 128:(dc + 1) * 128], ident[:]
                )
                nc.vector.tensor_copy(out1T[:, dc, qc * 128:(qc + 1) * 128], ptr[:])

        # ---- phase 4: MLP + residual + global LN2 --------------------
        W1_v = W1_d.rearrange("(dc p) f -> dc p f", p=128)
        W1_s = []
        for j in range(4):
            t = big8.tile([128, F], bf16, tag="big8")
            nc.gpsimd.dma_start(t[:], W1_v[j])
            W1_s.append(t)
        W2_v = W2_d.rearrange("(g fc p) d -> g p fc d", p=128, fc=4)
        W2_s = []
        for j in range(4):
            t = c8x.tile([128, 4, D], bf16, tag="c8x")
            nc.gpsimd.dma_start(t[:], W2_v[j])
            W2_s.append(t)
        h1T = []
        for j in range(2):
            h1t_half = d16.tile([128, 8, R], bf16, tag="d16")
            h1T.append(h1t_half)
        for fm in range(16):
            ph = ps_mm.tile([128, R], f32, tag="mm")
            for dc in range(4):
                nc.tensor.matmul(
                    ph[:],
                    lhsT=W1_s[dc][:, fm * 128:(fm + 1) * 128],
                    rhs=out1T[:, dc, :],
                    start=(dc == 0), stop=(dc == 3),
                )
            nc.scalar.activation(
                h1T[fm // 8][:, fm % 8, :], ph[:], AF.Relu, bias=b1s[:, fm:fm + 1]
            )
        w = out1  # in place: w = out1 + out2
        for qc in range(4):
            po = ps_mm.tile([128, D], f32, tag="mm")
            for fm in range(16):
                nc.tensor.matmul(
                    po[:],
                    lhsT=h1T[fm // 8][:, fm % 8, qc * 128:(qc + 1) * 128],
                    rhs=W2_s[fm // 4][:, fm % 4, :],
                    start=(fm == 0), stop=False,
                )
            nc.tensor.matmul(
                po[:], lhsT=ones_row[:], rhs=b2_r[:], start=False, stop=True
            )
            nc.vector.tensor_tensor(w[:, qc, :], po[:], out1[:, qc, :], ALU.add)

        cout2 = stats_start(w, "b")
        # Vp fills the second AllReduce's latency window
        VTo = d16.tile([128, 4, R], bf16, tag="d16")
        own_proj_packed(VTo, w_qkv["v"], bvs2)
        wo_project_packed(VTo, vp_v)
        stb2 = stats_finish(cout2, "b")
        fin_s = c8.tile([128, 4, D], f32, tag="c8")
        ln_apply(fin_s, w, stb2, store_view=fin_v)

    split_waits(nc)
    return nc


_NC_CACHE = None


def _get_nc():
    global _NC_CACHE
    if _NC_CACHE is None:
        _NC_CACHE = build_nc()
    return _NC_CACHE


def kernel(**inputs):
    inp = {k: np.ascontiguousarray(np.asarray(v, dtype=np.float32))
           for k, v in inputs.items()}
    in_maps = []
    for c in range(N_CORES):
        rows = slice(c * R, (c + 1) * R)
        in_maps.append(dict(
            x=inp["x"], Wq=inp["Wq"], Wk=inp["Wk"], Wv=inp["Wv"],
            bq=inp["bq"], bk=inp["bk"], bv=inp["bv"],
            Wo=inp["Wo"], bo=inp["bo"], W1=inp["W1"], b1=inp["b1"],
            W2=inp["W2"], b2=inp["b2"],
            x_rows=inp["x"][rows],
            ln_g_rows=inp["ln_g"][rows], ln_b_rows=inp["ln_b"][rows],
        ))
    nc = _get_nc()
    res = run_bass_kernel_spmd(nc, in_maps, list(range(N_CORES)))
    final = np.concatenate([res.results[c]["final_rows"] for c in range(N_CORES)])
    Kp = np.concatenate([res.results[c]["Kp_rows"] for c in range(N_CORES)])
    Vp = np.concatenate([res.results[c]["Vp_rows"] for c in range(N_CORES)])
    return (final, Kp, Vp)
